# revision 1
# baseline (speedup 1.0000x reference)
"""Trainium2 Bass kernel for nn_Copy_56470230008202 (sparse_attention).

Strategy (8 NeuronCores, SPMD, one launch):
  The reference's `mixh.reshape(1,-1,H)` / `q2 = qh.transpose(1,0,2,3).reshape(-1,1,H)`
  views scramble rows so that output row l' = n*128 + pg (head n, position
  group pg) draws features from positions t = pg*16 + j of head n only.
  Hence: core i owns heads {2i, 2i+1} == output rows [i*256, (i+1)*256).

  - conv0 (CIN->H, k=3): channel-sharded; core i computes x0 channels
    [128i, 128i+128) over all L from the (replicated) input o.
  - AllGather x0 (~55-140us, ring-bound); ~280 warm-up matmuls on resident
    kv data ride through the barrier so the PE HAM governor stays un-
    throttled (a PE-idle window re-throttles the clock to 1.2 GHz and the
    following phases run 2x slow until a long dense burst re-warms it).
  - conv1 (H->H, k=3): core i computes only its 128 q-channels (2 heads).
  - attention per tb of 512 q-columns, both heads: scores -> exp (ACT,
    the phase pacer) -> mix, software-pipelined one tb ahead.  Softmax
    denominator via an appended ones-column in kv (no max subtraction --
    scores are in [-6, 6]).  The normalization chain is kept OFF the mix-
    psum release path: denom row -> SBUF, PE outer-product broadcast (no
    gpsimd, whose FIFO would head-of-line block), partition-parallel
    reciprocal on the broadcast [64,512] (a [1,512] reciprocal is single-
    lane and costs 3.3us), then 16 strided multiplies write the scrambled
    cat tiles and free the psum.
  - out-proj with both heads merged in the free dim (N=256); catq
    contraction chain first so it overlaps the last tb's norm tail.
  - V/C logits vs full VC^T streamed in contiguous 520 KB blocks
    (66.6 MB/core; prefetch + steady ~250-310 GB/s), 4-bank psum per
    (group, head), evacuated bf16 by DVE, written out via SWDGE.
  All matmuls bf16 inputs / fp32 PSUM accumulation (fp8 was measured at
  2.7-3.8% rel err on the V/C GEMM -- over the 2e-2 gate -- so bf16 stays).
  Weight-norm, selu(f), transposes, per-tile contiguous repacking,
  sharding and the final bias add run on host. All DRAM tensors are packed
  so every DMA is a single large contiguous block (descriptor-efficient).
"""

import os
import sys

for _p in ("/opt/trn_rl_repo", "/root/.axon_site/_ro/trn_rl_repo"):
    if os.path.isdir(_p) and _p not in sys.path:
        sys.path.append(_p)

import numpy as np
import ml_dtypes

import concourse.bass as bass
import concourse.mybir as mybir
from concourse import bacc
from concourse.tile import TileContext
from concourse.bass_utils import run_bass_kernel_spmd

F32 = mybir.dt.float32
BF16 = mybir.dt.bfloat16
ALU = mybir.AluOpType
ACTF = mybir.ActivationFunctionType

H, NH, HD = 1024, 16, 64
CIN, VOCAB, LIMIT, L, S = 1280, 32000, 512, 2048, 2048
VC = VOCAB + LIMIT              # 32512 = 16 groups * 2032 = 64 * 508
NVB, VBW = 64, 508
NG, GW = 16, 2032               # V-stream groups: 4 vocab blocks per group
NCORES = 8
LAM, ALPHA = 1.0507009873554805, 1.6732632423543772


def _selu_from_psum(nc, tmp, psum_ap, bias_ap, out_ap, P, N, idx, pbase=0,
                    zeros=None):
    """out = selu(z) given psum = LAM*z (lambda folded into weights+bias).
    selu(z) = max(y,0) + LAM*ALPHA*(exp(min(y,0)/LAM) - 1),  y = LAM*z + b'.
    pbase: base partition of bias_ap -- SBUF operands of one instruction
    must share their base partition (walrus NCC_IBIR297).
    """
    m = tmp.tile([P, N], F32, name=f"selu_m{idx}", tag=f"selu_m{P}x{N}")
    r = tmp.tile([P, N], F32, name=f"selu_r{idx}", tag=f"selu_r{P}x{N}")
    e = tmp.tile([P, N], F32, name=f"selu_e{idx}", tag=f"selu_e{P}x{N}")
    t = tmp.tile([P, N], F32, name=f"selu_t{idx}", tag=f"selu_t{P}x{N}")
    z = zeros[pbase:pbase + P, :N]
    nc.vector.scalar_tensor_tensor(m, psum_ap, bias_ap, z, op0=ALU.add, op1=ALU.min)
    nc.vector.scalar_tensor_tensor(r, psum_ap, bias_ap, z, op0=ALU.add, op1=ALU.max)
    nc.scalar.activation(e, m, ACTF.Exp, scale=1.0 / LAM)
    nc.vector.tensor_scalar(t, e, LAM * ALPHA, -LAM * ALPHA, op0=ALU.mult, op1=ALU.add)
    nc.vector.tensor_tensor(out_ap, t, r, op=ALU.add)


def build_program():
    nc = bacc.Bacc("TRN2", target_bir_lowering=False, debug=False,
                   num_devices=NCORES)
    # all inputs packed per-SBUF-tile contiguous (column blocks)
    oTp = nc.declare_dram_parameter("oTp", [128, 10 * (L + 2)], BF16, isOutput=False)
    w0p = nc.declare_dram_parameter("w0p", [128, 3840], BF16, isOutput=False)
    w1p = nc.declare_dram_parameter("w1p", [128, 3072], BF16, isOutput=False)
    kvp = nc.declare_dram_parameter("kvp", [128, S], BF16, isOutput=False)
    kvagp = nc.declare_dram_parameter("kvagp", [128, 16 * 130], BF16, isOutput=False)
    wop = nc.declare_dram_parameter("wop", [128, 16 * 1024], BF16, isOutput=False)
    cst = nc.declare_dram_parameter("cst", [128, 10], F32, isOutput=False)
    vctp = nc.declare_dram_parameter("vctp", [NG, 8, 128, GW], BF16, isOutput=False)
    out = nc.declare_dram_parameter("out", [2, NG, 128, GW], BF16, isOutput=True)

    with TileContext(nc) as tc:
        _emit(tc, oTp, w0p, w1p, kvp, kvagp, wop, cst, vctp, out)
    if not nc.is_finalized():
        nc.finalize()
    return nc


def _emit(tc, oTp, w0p, w1p, kvp, kvagp, wop, cst, vctp, out):
    nc = tc.nc

    with tc.tile_pool(name="const", bufs=1) as constp, \
         tc.tile_pool(name="persist", bufs=1) as pers, \
         tc.tile_pool(name="dram", bufs=1, space="DRAM") as dram:
        zeros = constp.tile([128, 512], F32)
        nc.vector.memset(zeros, 0.0)
        cst_sb = constp.tile([128, 10], F32)
        nc.sync.dma_start(out=cst_sb, in_=cst[:, :])
        q0b_sb = cst_sb[:, 0:1]
        q1b_sb = cst_sb[:, 1:2]
        outb_sb = cst_sb[:, 2:10]

        # persistent activations; q holds both heads stacked [hh*64+d, t]
        q_sb = pers.tile([128, L], BF16)
        # cat tiles: one [128, 8*256] tile each; block kk covers cat-channel
        # chunk kk with col = kk*256 + hh*128 + l', so the scramble writes
        # collapse to one multi-dim-AP DVE op per (head, jj) instead of 16.
        catm = pers.tile([128, 8 * 256], BF16, name="catm")
        catq = pers.tile([128, 8 * 256], BF16, name="catq")
        aoT = [pers.tile([128, 256], BF16, name=f"aoT{m}") for m in range(8)]
        kvT_sb = pers.tile([128, S], BF16)
        kvag_sb = pers.tile([128, 16 * 130], BF16)

        # warm-up psum lives in its own outer pool so neither conv1 nor the
        # attention pools inherit a false dependency on the warm matmuls.
        with tc.tile_pool(name="warmps", bufs=1, space="PSUM") as wps:
            # ---------------- conv0 + AllGather ----------------
            with tc.tile_pool(name="c0", bufs=1) as c0p, \
                 tc.tile_pool(name="c0ps", bufs=3, space="PSUM") as c0ps, \
                 tc.tile_pool(name="c0tmp", bufs=2) as c0tmp:
                # chunked loads so the first matmuls start early
                oT_sb = c0p.tile([128, 10 * (L + 2)], BF16)
                w0_sb = c0p.tile([128, 3840], BF16)
                nc.sync.dma_start(out=w0_sb[:, 0:1280], in_=w0p[:, 0:1280])
                nc.sync.dma_start(out=oT_sb[:, 0:L + 2], in_=oTp[:, 0:L + 2])
                for k in range(1, 3):
                    nc.sync.dma_start(out=w0_sb[:, k * 1280:(k + 1) * 1280],
                                      in_=w0p[:, k * 1280:(k + 1) * 1280])
                for c in range(1, 10):
                    nc.sync.dma_start(
                        out=oT_sb[:, c * (L + 2):(c + 1) * (L + 2)],
                        in_=oTp[:, c * (L + 2):(c + 1) * (L + 2)])
                x0loc = c0p.tile([128, L], BF16)
                x0src1 = dram.tile([128, L], BF16, name="x0src1")
                x0g1 = dram.tile([H, L], BF16, name="x0g1", addr_space="Shared")
                for tb in range(4):
                    ps = c0ps.tile([128, 512], F32, name="c0psum", tag="c0psum")
                    idx = 0
                    for c in range(10):
                        for k in range(3):
                            nc.tensor.matmul(
                                ps, lhsT=w0_sb[:, (k * 10 + c) * 128:
                                               (k * 10 + c + 1) * 128],
                                rhs=oT_sb[:, c * (L + 2) + tb * 512 + k:
                                          c * (L + 2) + tb * 512 + k + 512],
                                start=(idx == 0), stop=(idx == 29))
                            idx += 1
                    dst = x0loc[:, tb * 512:(tb + 1) * 512]
                    _selu_from_psum(nc, c0tmp, ps, q0b_sb, dst, 128, 512,
                                    f"c0_{tb}", zeros=zeros)
                    # stage the collective's source progressively so the
                    # gather launches right after the last chunk
                    nc.sync.dma_start(out=x0src1[:, tb * 512:(tb + 1) * 512],
                                      in_=dst)
                nc.gpsimd.collective_compute(
                    "AllGather", ALU.bypass,
                    replica_groups=[list(range(NCORES))],
                    ins=[x0src1.opt()], outs=[x0g1.opt()])
                # attention loads fill the DMA queues during the gather
                nc.sync.dma_start(out=kvT_sb, in_=kvp[:, :])
                nc.sync.dma_start(out=kvag_sb, in_=kvagp[:, :])

            # Warm-up matmuls: keep the PE busy (and HAM at K=8/8) through
            # the ~60us AllGather barrier.  They read the persistent kv tile
            # (arrives ~2us into the gather) and recycle one scratch psum
            # bank; nothing downstream depends on them.
            warm = wps.tile([128, 512], F32, name="warm", tag="warm")
            for _ in range(280):
                nc.tensor.matmul(warm, lhsT=kvT_sb[:, 0:128],
                                 rhs=kvT_sb[:, 0:512], start=True, stop=True)

            # ---------------- conv1 ----------------
            with tc.tile_pool(name="c1", bufs=1) as c1p, \
                 tc.tile_pool(name="c1ps", bufs=3, space="PSUM") as c1ps, \
                 tc.tile_pool(name="c1tmp", bufs=2) as c1tmp:
                w1_sb = c1p.tile([128, 3072], BF16)
                nc.sync.dma_start(out=w1_sb, in_=w1p[:, :])
                x0f = [c1p.tile([128, L + 2], BF16, name=f"x0g{c}")
                       for c in range(8)]
                for c in range(8):
                    nc.vector.memset(x0f[c][:, 0:1], 0.0)
                    nc.vector.memset(x0f[c][:, L + 1:L + 2], 0.0)
                    nc.gpsimd.dma_start(
                        out=x0f[c][:, 1:L + 1],
                        in_=x0g1[c * 128:(c + 1) * 128, :])
                for tb in range(4):
                    ps = c1ps.tile([128, 512], F32, name="c1psum", tag="c1psum")
                    idx = 0
                    for k in range(3):
                        for c in range(8):
                            nc.tensor.matmul(
                                ps, lhsT=w1_sb[:, (k * 8 + c) * 128:
                                               (k * 8 + c + 1) * 128],
                                rhs=x0f[c][:, tb * 512 + k: tb * 512 + k + 512],
                                start=(idx == 0), stop=(idx == 23))
                            idx += 1
                    _selu_from_psum(nc, c1tmp, ps, q1b_sb,
                                    q_sb[:, tb * 512:(tb + 1) * 512],
                                    128, 512, f"c1_{tb}", zeros=zeros)

        # ------------- attention + scramble -------------
        with tc.tile_pool(name="wo", bufs=1) as wop_:
            wo_sb = wop_.tile([128, 16 * 1024], BF16)
            nc.sync.dma_start(out=wo_sb, in_=wop[:, :])

            with tc.tile_pool(name="attn", bufs=1) as atp, \
                 tc.tile_pool(name="ppool", bufs=16) as ppool, \
                 tc.tile_pool(name="dnmp", bufs=2) as dnmp, \
                 tc.tile_pool(name="rbcp", bufs=2) as rbcp, \
                 tc.tile_pool(name="scps", bufs=2, space="PSUM") as scps, \
                 tc.tile_pool(name="mixps", bufs=2, space="PSUM") as mixps, \
                 tc.tile_pool(name="bcps", bufs=1, space="PSUM") as bcps, \
                 tc.tile_pool(name="fillps", bufs=1, space="PSUM") as fillps:
                ones_sb = atp.tile([128, 64], BF16)
                nc.vector.memset(ones_sb, 1.0)
                fill_ps = fillps.tile([128, 512], F32, name="fill", tag="fill")

                def filler():
                    # surplus PE work with zero cross-engine deps: keeps the
                    # PE saturated at any clock >= 13/16 so the HAM governor
                    # never sees an idle window and re-throttles to K=4/8
                    # (the 13/16 <-> 4/8 limit cycle cost ~55us in attention)
                    nc.tensor.matmul(fill_ps, lhsT=kvT_sb[:, 0:128],
                                     rhs=kvT_sb[:, 0:512],
                                     start=True, stop=True)
                # q col = pg*16 + kk*2 + jj; one copy per (head, jj)
                qre = q_sb.rearrange("p (pg kk jj) -> p jj kk pg", kk=8, jj=2)
                cqre = catq.rearrange("p (kk c) -> p kk c", c=256)
                for hh in range(2):
                    for jj in range(2):
                        nc.vector.tensor_copy(
                            out=cqre[jj * 64:(jj + 1) * 64, :,
                                     hh * 128:(hh + 1) * 128],
                            in_=qre[hh * 64:(hh + 1) * 64, jj, :, :])

                def emit_scores(tb, nfill):
                    # both heads of one tb; two s'-tiles per 2-bank psum so a
                    # single exp covers 1024 columns (amortizes ACT per-op
                    # overhead).  nfill filler matmuls per (st2, head) unit
                    # absorb the ACT-pacing slack.
                    plist = [[], []]
                    for st2 in range(8):
                        for hh in range(2):
                            ps2 = scps.tile([128, 1024], F32, name="ps_sc",
                                            tag="ps_sc")
                            for half in range(2):
                                st = 2 * st2 + half
                                nc.tensor.matmul(
                                    ps2[:, half * 512:(half + 1) * 512],
                                    lhsT=kvT_sb[hh * 64:(hh + 1) * 64,
                                                st * 128:(st + 1) * 128],
                                    rhs=q_sb[hh * 64:(hh + 1) * 64,
                                             tb * 512:(tb + 1) * 512],
                                    start=True, stop=True)
                            p2 = ppool.tile([128, 1024], BF16, name="p_t",
                                            tag="p")
                            nc.scalar.activation(p2, ps2, ACTF.Exp, scale=0.125)
                            plist[hh].append(p2[:, 0:512])
                            plist[hh].append(p2[:, 512:1024])
                            for _ in range(nfill):
                                filler()
                    return plist

                def emit_mix(tb, plist):
                    # 32 back-to-back matmuls: a ~7us dense PE burst every tb
                    pms = []
                    for hh in range(2):
                        ps_mix = mixps.tile([65, 512], F32, name="ps_mix",
                                            tag="ps_mix")
                        for st in range(16):
                            nc.tensor.matmul(
                                ps_mix,
                                lhsT=kvag_sb[:, st * 130 + hh * 65:
                                             st * 130 + (hh + 1) * 65],
                                rhs=plist[hh][st][:, :],
                                start=(st == 0), stop=(st == 15))
                        pms.append(ps_mix)
                    return pms

                def emit_norm(tb, pms):
                    # normalization chain kept OFF the psum-release critical
                    # path: denom row -> SBUF (bf16), PE outer-product
                    # broadcast (no gpsimd), partition-parallel reciprocal,
                    # then the 16 scramble multiplies free the mix psum.
                    dnms, bcs, rbcs = [], [], []
                    for hh in range(2):
                        dnm = dnmp.tile([128, 512], BF16, name="dnm", tag="dnm")
                        nc.vector.tensor_copy(out=dnm[64:65, :],
                                              in_=pms[hh][64:65, :])
                        dnms.append(dnm)
                    filler()
                    filler()
                    for hh in range(2):
                        bc = bcps.tile([64, 512], F32, name="bc", tag="bc")
                        nc.tensor.matmul(bc, lhsT=ones_sb[64:65, :],
                                         rhs=dnms[hh][64:65, :],
                                         start=True, stop=True)
                        bcs.append(bc)
                    for hh in range(2):
                        rbc = rbcp.tile([64, 512], F32, name="rbc", tag="rbc")
                        nc.vector.reciprocal(rbc, bcs[hh])
                        rbcs.append(rbc)
                    cmre = catm.rearrange("p (kk c) -> p kk c", c=256)
                    for hh in range(2):
                        mre = pms[hh][0:64, :].rearrange(
                            "p (pg kk jj) -> p jj kk pg", kk=8, jj=2)
                        rre = rbcs[hh].rearrange(
                            "p (pg kk jj) -> p jj kk pg", kk=8, jj=2)
                        col = hh * 128 + tb * 32
                        for jj in range(2):
                            nc.vector.tensor_tensor(
                                out=cmre[jj * 64:(jj + 1) * 64, :,
                                         col:col + 32],
                                in0=mre[:, jj, :, :],
                                in1=rre[:, jj, :, :],
                                op=ALU.mult)

                # software pipeline: scores(tb+1) (ACT-paced) is emitted
                # before mix(tb); the norm chain trails one step behind.
                # The prologue tb has no mix stream to ride on, so it gets
                # extra filler.
                pl = {0: emit_scores(0, 2)}
                for tb in range(4):
                    if tb + 1 < 4:
                        pl[tb + 1] = emit_scores(tb + 1, 1)
                    pms = emit_mix(tb, pl.pop(tb))
                    emit_norm(tb, pms)

            # ---- out-projection + V/C logits share one 8-bank psum pool ----
            with tc.tile_pool(name="vstream", bufs=24) as vsp, \
                 tc.tile_pool(name="vstage", bufs=6) as vst, \
                 tc.tile_pool(name="otmp", bufs=2) as otmp, \
                 tc.tile_pool(name="vps", bufs=2, space="PSUM") as vps:
                for m in range(8):
                    ps_f = vps.tile([128, 2048], F32, name="ps_v", tag="ps_v")
                    ps_o = ps_f[:, 0:256]
                    # catq chain first: it is ready at conv1 time, so these
                    # matmuls overlap the last tb's norm-chain tail on DVE.
                    for k in range(8):
                        nc.tensor.matmul(
                            ps_o,
                            lhsT=wo_sb[:, (8 + k) * 1024 + m * 128:
                                       (8 + k) * 1024 + (m + 1) * 128],
                            rhs=catq[:, k * 256:(k + 1) * 256],
                            start=(k == 0), stop=False)
                    for k in range(8):
                        nc.tensor.matmul(
                            ps_o,
                            lhsT=wo_sb[:, k * 1024 + m * 128:
                                       k * 1024 + (m + 1) * 128],
                            rhs=catm[:, k * 256:(k + 1) * 256],
                            start=False, stop=(k == 7))
                    _selu_from_psum(nc, otmp, ps_o, outb_sb[:, m:m + 1],
                                    aoT[m][:, :], 128, 256,
                                    f"o_{m}", zeros=zeros)

                for g in range(NG):
                    vtiles = []
                    for k in range(8):
                        vt = vsp.tile([128, GW], BF16, name="vt", tag="vct")
                        nc.sync.dma_start(out=vt, in_=vctp[g, k, :, :])
                        vtiles.append(vt)
                    for hh in range(2):
                        stg = vst.tile([128, GW], BF16, name="vstage",
                                       tag="vstage")
                        # one 4-bank psum per (g, hh): each 508-wide matmul
                        # sits inside its own bank (512-aligned); a single
                        # strided copy evacuates all 4 -- one op instead of
                        # four.
                        ps4 = vps.tile([128, 2048], F32, name="ps_v", tag="ps_v")
                        for u in range(4):
                            for k in range(8):
                                nc.tensor.matmul(
                                    ps4[:, u * 512: u * 512 + VBW],
                                    lhsT=aoT[k][:, hh * 128:(hh + 1) * 128],
                                    rhs=vtiles[k][:, u * VBW:(u + 1) * VBW],
                                    start=(k == 0), stop=(k == 7))
                        src = ps4.rearrange("p (u w) -> p u w", w=512)[:, :, 0:VBW]
                        dst = stg.rearrange("p (u w) -> p u w", w=VBW)
                        # DVE evacuates (ACT is the attention pacer); the
                        # output writes go out the SWDGE ring so their
                        # stage-copy gating never stalls the in-order sync
                        # ring streaming vct.
                        nc.vector.tensor_copy(out=dst, in_=src)
                        nc.gpsimd.dma_start(out=out[hh, g, :, :], in_=stg)


# ---------------- host side ----------------

def _wn_conv(v, g):
    n = np.sqrt((v * v).sum(axis=(1, 2), keepdims=True))
    return g[:, None, None] * v / n


def _wn_lin(v, g):
    return g[:, None] * v / np.linalg.norm(v, axis=1, keepdims=True)


def _selu_np(x):
    return np.where(x > 0, LAM * x,
                    LAM * ALPHA * (np.exp(np.minimum(x, 0)) - 1)).astype(np.float32)


def _bf16(x):
    return np.ascontiguousarray(x.astype(ml_dtypes.bfloat16))


def _f32(x):
    return np.ascontiguousarray(x.astype(np.float32))


_PROGRAM_CACHE = {}


def kernel(o, f, q0_v, q0_g, q0_b, q1_v, q1_g, q1_b,
           out_v, out_g, out_b, V_v, V_g, V_b, C_v, C_g, C_b):
    o, f = np.asarray(o), np.asarray(f)

    w0 = _wn_conv(np.asarray(q0_v), np.asarray(q0_g)) * LAM      # (H, CIN, 3)
    w1 = _wn_conv(np.asarray(q1_v), np.asarray(q1_g)) * LAM      # (H, H, 3)
    b0 = np.asarray(q0_b) * LAM
    b1 = np.asarray(q1_b) * LAM
    woutT = np.ascontiguousarray(_wn_lin(np.asarray(out_v), np.asarray(out_g)).T) * LAM
    outb_l = np.asarray(out_b) * LAM
    vc = np.concatenate([_wn_lin(np.asarray(V_v), np.asarray(V_g)),
                         _wn_lin(np.asarray(C_v), np.asarray(C_g))], axis=0)
    vct = np.ascontiguousarray(vc.T)                             # (H, 32512)
    kv = _selu_np(f)                                             # (S, H)

    # packed layouts (every SBUF tile contiguous in DRAM)
    oT_pad = np.zeros((CIN, L + 2), np.float32)
    oT_pad[:, 1:L + 1] = o.T
    oTp = _bf16(oT_pad.reshape(10, 128, L + 2).transpose(1, 0, 2)
                .reshape(128, 10 * (L + 2)))
    w0T = w0.transpose(2, 1, 0).reshape(3 * CIN, H)              # (3840, 1024)
    w1T = w1.transpose(2, 1, 0).reshape(3 * H, H)                # (3072, 1024)
    wopk = _bf16(woutT.reshape(16, 128, 1024).transpose(1, 0, 2)
                 .reshape(128, 16 * 1024))
    vctp = _bf16(vct.reshape(8, 128, NG, GW).transpose(2, 0, 1, 3))
    kvT_full = np.ascontiguousarray(kv.T)                        # (H, S)

    if "nc" not in _PROGRAM_CACHE:
        _PROGRAM_CACHE["nc"] = build_program()
    nc = _PROGRAM_CACHE["nc"]

    in_maps = []
    for i in range(NCORES):
        sl = slice(i * 128, (i + 1) * 128)
        kvag = np.zeros((S, 130), np.float32)
        for hh in range(2):
            n = 2 * i + hh
            kvag[:, hh * 65:hh * 65 + 64] = kv[:, n * 64:(n + 1) * 64]
            kvag[:, hh * 65 + 64] = 1.0
        kvagp = _bf16(kvag.reshape(16, 128, 130).transpose(1, 0, 2)
                      .reshape(128, 16 * 130))
        w0pi = _bf16(w0T[:, sl].reshape(30, 128, 128).transpose(1, 0, 2)
                     .reshape(128, 3840))
        w1pi = _bf16(w1T[:, sl].reshape(24, 128, 128).transpose(1, 0, 2)
                     .reshape(128, 3072))
        kvpi = _bf16(kvT_full[sl, :])
        csti = np.zeros((128, 10), np.float32)
        csti[:, 0] = b0[sl]
        csti[:, 1] = b1[sl]
        csti[:, 2:10] = outb_l.reshape(8, 128).T
        in_maps.append({
            "oTp": oTp,
            "w0p": w0pi,
            "w1p": w1pi,
            "kvp": kvpi,
            "kvagp": kvagp,
            "wop": wopk,
            "cst": _f32(csti),
            "vctp": vctp,
        })

    kwargs = {}
    if os.environ.get("NN_COPY_TRACE", "0") == "1":
        kwargs = dict(trace=True)
    res = run_bass_kernel_spmd(nc, in_maps, core_ids=list(range(NCORES)), **kwargs)
    global LAST_RESULTS
    LAST_RESULTS = res
    shards = []
    for i in range(NCORES):
        od = np.asarray(res.results[i]["out"]).astype(np.float32)  # (2,NG,128,GW)
        shards.append(od.transpose(0, 2, 1, 3).reshape(256, VC))
    full = np.concatenate(shards, axis=0)                        # (2048, 32512)
    full += np.concatenate([np.asarray(V_b), np.asarray(C_b)])[None, :]
    return full



# revision 9
# speedup vs baseline: 1.2270x; 1.2270x over previous
"""Trainium2 Bass kernel for nn_Copy_56470230008202 (sparse_attention).

Strategy (8 NeuronCores, SPMD, one launch) -- v2: collective-free L-sharding.

  The reference's `mixh.reshape(1,-1,H)` / `q2 = qh.transpose(1,0,2,3)`
  views scramble rows so that output row r = n*128 + pg (head n, position
  group pg) draws ONLY from q positions t = pg*16 + j (j=0..15) of head n.
  Hence a core that owns a contiguous 256-slice of L -- q positions
  [256i, 256i+256), i.e. pg in [16i, 16i+16) -- can compute 256 COMPLETE
  output rows {r = n*128 + 16i + pgl} for ALL 16 heads with NO collective:

  - conv0/conv1 L-sharded: each core computes all 1024 channels over its
    258/256 local columns (1-col halos computed redundantly; the host
    pads oT with the 2 extra input columns).  Full w0/w1 are streamed
    (7.5+6 MB, hidden under conv PE time).  This removes the barrier +
    AllGather (~80us dead window in v1) and the 280 warm-up matmuls.
  - attention in 8 waves of (head pair w, 256 local q-cols): scores
    N=256 with the two heads at PE row-groups 0/64 (concurrent
    sub-array execution), exp on [128,1024] psums (ACT is the wave
    pacer at ~9.2us/wave), mix into [65,256] psums with an appended
    ones-column for the softmax denominator, then the same
    PE-broadcast + partition-parallel-reciprocal + strided-multiply
    norm chain as v1, writing the scrambled cat tiles per (head, jlo).
  - out-proj and the V/C logits stream are IDENTICAL to v1: both heads
    merged in the free dim (N=256), then 16 groups x 2 row-halves of
    vct streamed in contiguous 520 KB blocks, 4-bank psum, DVE
    evacuation, SWDGE writes out.  That phase is at the PE roofline
    (1 col/cycle at the GPIO-throttled 13/16 clock) and cannot be
    made faster by resharding -- v2 instead removes the ~80us
    collective window and ~45us of attention-phase slack in front
    of it.
  All matmuls bf16 inputs / fp32 PSUM accumulation.  Weight-norm,
  selu(f), transposes, per-tile contiguous repacking, sharding and the
  final bias add run on host.  All DRAM tensors are packed so every DMA
  is a single large contiguous block.
"""

import os
import sys

for _p in ("/opt/trn_rl_repo", "/root/.axon_site/_ro/trn_rl_repo"):
    if os.path.isdir(_p) and _p not in sys.path:
        sys.path.append(_p)

import numpy as np
import ml_dtypes

import concourse.bass as bass
import concourse.mybir as mybir
from concourse import bacc
from concourse.tile import TileContext
from concourse.bass_utils import run_bass_kernel_spmd

F32 = mybir.dt.float32
BF16 = mybir.dt.bfloat16
ALU = mybir.AluOpType
ACTF = mybir.ActivationFunctionType

H, NH, HD = 1024, 16, 64
CIN, VOCAB, LIMIT, L, S = 1280, 32000, 512, 2048, 2048
VC = VOCAB + LIMIT              # 32512 = 16 groups * 2032 = 64 * 508
NVB, VBW = 64, 508
NG, GW = 16, 2032               # V-stream groups: 4 vocab blocks per group
NCORES = 8
LQ = L // NCORES                # 256 local q columns per core
LAM, ALPHA = 1.0507009873554805, 1.6732632423543772


def _selu_from_psum(nc, tmp, psum_ap, bias_ap, out_ap, P, N, idx, pbase=0,
                    zeros=None):
    """out = selu(z) given psum = LAM*z (lambda folded into weights+bias).
    selu(z) = max(y,0) + LAM*ALPHA*(exp(min(y,0)/LAM) - 1),  y = LAM*z + b'.
    """
    m = tmp.tile([P, N], F32, name=f"selu_m{idx}", tag=f"selu_m{P}x{N}")
    r = tmp.tile([P, N], F32, name=f"selu_r{idx}", tag=f"selu_r{P}x{N}")
    e = tmp.tile([P, N], F32, name=f"selu_e{idx}", tag=f"selu_e{P}x{N}")
    t = tmp.tile([P, N], F32, name=f"selu_t{idx}", tag=f"selu_t{P}x{N}")
    z = zeros[pbase:pbase + P, :N]
    nc.vector.scalar_tensor_tensor(m, psum_ap, bias_ap, z, op0=ALU.add, op1=ALU.min)
    nc.vector.scalar_tensor_tensor(r, psum_ap, bias_ap, z, op0=ALU.add, op1=ALU.max)
    nc.scalar.activation(e, m, ACTF.Exp, scale=1.0 / LAM)
    nc.vector.tensor_scalar(t, e, LAM * ALPHA, -LAM * ALPHA, op0=ALU.mult, op1=ALU.add)
    nc.vector.tensor_tensor(out_ap, t, r, op=ALU.add)


def build_program():
    nc = bacc.Bacc("TRN2", target_bir_lowering=False, debug=False,
                   num_devices=NCORES)
    # all inputs packed per-SBUF-tile contiguous (column blocks)
    oTp = nc.declare_dram_parameter("oTp", [128, 10 * 260], BF16, isOutput=False)
    w0p = nc.declare_dram_parameter("w0p", [128, 30 * 1024], BF16, isOutput=False)
    w1p = nc.declare_dram_parameter("w1p", [128, 24 * 1024], BF16, isOutput=False)
    kvtp = nc.declare_dram_parameter("kvtp", [128, 8 * 2048], BF16, isOutput=False)
    kvagp = nc.declare_dram_parameter("kvagp", [128, 8 * 2080], BF16, isOutput=False)
    wop = nc.declare_dram_parameter("wop", [128, 16 * 1024], BF16, isOutput=False)
    cst = nc.declare_dram_parameter("cst", [128, 26], F32, isOutput=False)
    vctp = nc.declare_dram_parameter("vctp", [NG, 8, 128, GW], BF16, isOutput=False)
    out = nc.declare_dram_parameter("out", [2, NG, 128, GW], BF16, isOutput=True)

    with TileContext(nc) as tc:
        _emit(tc, oTp, w0p, w1p, kvtp, kvagp, wop, cst, vctp, out)
    if not nc.is_finalized():
        nc.finalize()
    return nc


def _emit(tc, oTp, w0p, w1p, kvtp, kvagp, wop, cst, vctp, out):
    nc = tc.nc

    with tc.tile_pool(name="const", bufs=1) as constp, \
         tc.tile_pool(name="persist", bufs=1) as pers:
        zeros = constp.tile([128, 512], F32)
        nc.vector.memset(zeros, 0.0)
        cst_sb = constp.tile([128, 26], F32)
        nc.sync.dma_start(out=cst_sb, in_=cst[:, :])

        # persistent activations
        q_sb = pers.tile([128, 8 * LQ], BF16)         # conv1 out, chunk m cols
        catm = pers.tile([128, 8 * LQ], BF16, name="catm")
        catq = pers.tile([128, 8 * LQ], BF16, name="catq")
        aoT = [pers.tile([128, LQ], BF16, name=f"aoT{m}") for m in range(8)]

        with tc.tile_pool(name="kvp", bufs=1) as kvp:
            kvt_sb = kvp.tile([128, 8 * 2048], BF16)   # [hd ch, s] per pair
            kvag_sb = kvp.tile([128, 8 * 2080], BF16)  # [s,(st,hh,65)] per pair

            # ---------------- conv0 / conv1 ----------------
            with tc.tile_pool(name="c0", bufs=1) as c0p:
                oT_sb = c0p.tile([128, 10 * 260], BF16)
                x0 = c0p.tile([128, 8 * 258], BF16)
                with tc.tile_pool(name="c0w", bufs=1) as c0w, \
                     tc.tile_pool(name="c0ps", bufs=1, space="PSUM") as c0ps, \
                     tc.tile_pool(name="c0tmp", bufs=3) as c0tmp:
                    w0_sb = c0w.tile([128, 30 * 1024], BF16)
                    nc.sync.dma_start(out=oT_sb, in_=oTp[:, :])
                    for j in range(10):              # 3 kc-slices per DMA
                        nc.sync.dma_start(out=w0_sb[:, j * 3072:(j + 1) * 3072],
                                          in_=w0p[:, j * 3072:(j + 1) * 3072])
                    pss = [c0ps.tile([128, 258], F32, name=f"c0ps{m}",
                                     tag=f"c0ps{m}") for m in range(8)]
                    for kc in range(30):
                        k, c = kc // 10, kc % 10
                        for m in range(8):
                            nc.tensor.matmul(
                                pss[m],
                                lhsT=w0_sb[:, (kc * 8 + m) * 128:
                                           (kc * 8 + m + 1) * 128],
                                rhs=oT_sb[:, c * 260 + k: c * 260 + k + 258],
                                start=(kc == 0), stop=(kc == 29))
                    for m in range(8):
                        _selu_from_psum(nc, c0tmp, pss[m], cst_sb[:, m:m + 1],
                                        x0[:, m * 258:(m + 1) * 258], 128, 258,
                                        f"c0_{m}", zeros=zeros)
                    # halo columns at the global sequence edges are conv1's
                    # zero-pad positions: mask them (cores 0/7 get 0, else 1)
                    for m in range(8):
                        nc.vector.tensor_tensor(
                            out=x0[:, m * 258:m * 258 + 1],
                            in0=x0[:, m * 258:m * 258 + 1],
                            in1=cst_sb[:, 24:25], op=ALU.mult)
                        nc.vector.tensor_tensor(
                            out=x0[:, m * 258 + 257:m * 258 + 258],
                            in0=x0[:, m * 258 + 257:m * 258 + 258],
                            in1=cst_sb[:, 25:26], op=ALU.mult)

                with tc.tile_pool(name="c1w", bufs=1) as c1w, \
                     tc.tile_pool(name="c1ps", bufs=1, space="PSUM") as c1ps, \
                     tc.tile_pool(name="c1tmp", bufs=3) as c1tmp:
                    w1_sb = c1w.tile([128, 24 * 1024], BF16)
                    for j in range(8):               # 3 kc-slices per DMA
                        nc.sync.dma_start(out=w1_sb[:, j * 3072:(j + 1) * 3072],
                                          in_=w1p[:, j * 3072:(j + 1) * 3072])
                    # attention data behind the conv weights on the sync ring
                    for w in range(8):
                        nc.sync.dma_start(out=kvt_sb[:, w * 2048:(w + 1) * 2048],
                                          in_=kvtp[:, w * 2048:(w + 1) * 2048])
                        nc.sync.dma_start(out=kvag_sb[:, w * 2080:(w + 1) * 2080],
                                          in_=kvagp[:, w * 2080:(w + 1) * 2080])
                    ps1 = [c1ps.tile([128, LQ], F32, name=f"c1ps{m}",
                                     tag=f"c1ps{m}") for m in range(8)]
                    for kc in range(24):
                        k, c = kc // 8, kc % 8
                        for m in range(8):
                            nc.tensor.matmul(
                                ps1[m],
                                lhsT=w1_sb[:, (kc * 8 + m) * 128:
                                           (kc * 8 + m + 1) * 128],
                                rhs=x0[:, c * 258 + k: c * 258 + k + 256],
                                start=(kc == 0), stop=(kc == 23))
                    for m in range(8):
                        _selu_from_psum(nc, c1tmp, ps1[m], cst_sb[:, 8 + m:9 + m],
                                        q_sb[:, m * LQ:(m + 1) * LQ], 128, LQ,
                                        f"c1_{m}", zeros=zeros)

            # ------------- attention + scramble -------------
            with tc.tile_pool(name="wo", bufs=1) as wop_:
                wo_sb = wop_.tile([128, 16 * 1024], BF16)
                nc.sync.dma_start(out=wo_sb, in_=wop[:, :])

                with tc.tile_pool(name="attn", bufs=1) as atp, \
                     tc.tile_pool(name="ppool", bufs=8) as ppool, \
                     tc.tile_pool(name="dnmp", bufs=2) as dnmp, \
                     tc.tile_pool(name="rbcp", bufs=2) as rbcp, \
                     tc.tile_pool(name="scps", bufs=2, space="PSUM") as scps, \
                     tc.tile_pool(name="mixps", bufs=2, space="PSUM") as mixps, \
                     tc.tile_pool(name="bcps", bufs=2, space="PSUM") as bcps:
                    ones_sb = atp.tile([128, 64], BF16)
                    nc.vector.memset(ones_sb, 1.0)

                    # catq scramble: catq[jlo*64+d, kk*256 + n*16 + pgl]
                    #   = q_sb[m=n//2][(n%2)*64+d, pgl*16 + kk*2 + jlo]
                    cqre = catq.rearrange("p (kk c) -> p kk c", c=LQ)
                    for n in range(NH):
                        m, hh = n // 2, n % 2
                        qre = q_sb[:, m * LQ:(m + 1) * LQ].rearrange(
                            "p (pgl kk jlo) -> p jlo kk pgl", kk=8, jlo=2)
                        for jlo in range(2):
                            nc.vector.tensor_copy(
                                out=cqre[jlo * 64:(jlo + 1) * 64, :,
                                         n * 16:(n + 1) * 16],
                                in_=qre[hh * 64:(hh + 1) * 64, jlo, :, :])

                    def sc_psum(w, j):
                        # 4 score tiles (hh, st parity) in one 2-bank psum,
                        # heads at PE row-groups 0/64 run concurrently
                        ps = scps.tile([128, 1024], F32, name="ps_sc",
                                       tag="ps_sc")
                        for hh in range(2):
                            for par in range(2):
                                st = 2 * j + par
                                nc.tensor.matmul(
                                    ps[:, hh * 512 + par * 256:
                                       hh * 512 + (par + 1) * 256],
                                    lhsT=kvt_sb[hh * 64:(hh + 1) * 64,
                                                w * 2048 + st * 128:
                                                w * 2048 + (st + 1) * 128],
                                    rhs=q_sb[hh * 64:(hh + 1) * 64,
                                             w * LQ:(w + 1) * LQ],
                                    start=True, stop=True)
                        p = ppool.tile([128, 1024], BF16, name="p_t", tag="p")
                        nc.scalar.activation(p, ps, ACTF.Exp, scale=0.125)
                        return p

                    def mix_pair(w, j, plist, pms):
                        # accumulate st = 2j, 2j+1 for both heads
                        for hh in range(2):
                            for par in range(2):
                                st = 2 * j + par
                                nc.tensor.matmul(
                                    pms[hh],
                                    lhsT=kvag_sb[:,
                                                 w * 2080 + st * 130 + hh * 65:
                                                 w * 2080 + st * 130 + (hh + 1) * 65],
                                    rhs=plist[j][:, hh * 512 + par * 256:
                                                 hh * 512 + (par + 1) * 256],
                                    start=(st == 0), stop=(st == 15))

                    def emit_norm(w, pms):
                        dnms, bcs, rbcs = [], [], []
                        for hh in range(2):
                            dnm = dnmp.tile([128, LQ], BF16, name="dnm",
                                            tag="dnm")
                            nc.vector.tensor_copy(out=dnm[64:65, :],
                                                  in_=pms[hh][64:65, :])
                            dnms.append(dnm)
                        for hh in range(2):
                            bc = bcps.tile([64, LQ], F32, name="bc", tag="bc")
                            nc.tensor.matmul(bc, lhsT=ones_sb[64:65, :],
                                             rhs=dnms[hh][64:65, :],
                                             start=True, stop=True)
                            bcs.append(bc)
                        for hh in range(2):
                            rbc = rbcp.tile([64, LQ], F32, name="rbc",
                                            tag="rbc")
                            nc.vector.reciprocal(rbc, bcs[hh])
                            rbcs.append(rbc)
                        cmre = catm.rearrange("p (kk c) -> p kk c", c=LQ)
                        for hh in range(2):
                            n = 2 * w + hh
                            mre = pms[hh][0:64, :].rearrange(
                                "p (pgl kk jlo) -> p jlo kk pgl", kk=8, jlo=2)
                            rre = rbcs[hh].rearrange(
                                "p (pgl kk jlo) -> p jlo kk pgl", kk=8, jlo=2)
                            for jlo in range(2):
                                nc.vector.tensor_tensor(
                                    out=cmre[jlo * 64:(jlo + 1) * 64, :,
                                             n * 16:(n + 1) * 16],
                                    in0=mre[:, jlo, :, :],
                                    in1=rre[:, jlo, :, :],
                                    op=ALU.mult)

                    # software pipeline interleaved at j granularity:
                    # PE order per wave w: [mix(w,j); scores(w+1,j)] x8; norm(w)
                    plist = [sc_psum(0, j) for j in range(8)]
                    for w in range(8):
                        pms = [mixps.tile([65, 512], F32, name="ps_mix",
                                          tag="ps_mix")[:, 0:256]
                               for _ in range(2)]
                        nxt = []
                        for j in range(8):
                            mix_pair(w, j, plist, pms)
                            if w + 1 < 8:
                                nxt.append(sc_psum(w + 1, j))
                        emit_norm(w, pms)
                        plist = nxt

                # ---- out-projection (wo still resident) ----
                with tc.tile_pool(name="otmp", bufs=2) as otmp, \
                     tc.tile_pool(name="ops", bufs=2, space="PSUM") as ops:
                    for m in range(8):
                        ps_o = ops.tile([128, 256], F32, name="ps_o",
                                        tag="ps_o")
                        for k in range(8):
                            nc.tensor.matmul(
                                ps_o,
                                lhsT=wo_sb[:, (8 + k) * 1024 + m * 128:
                                           (8 + k) * 1024 + (m + 1) * 128],
                                rhs=catq[:, k * 256:(k + 1) * 256],
                                start=(k == 0), stop=False)
                        for k in range(8):
                            nc.tensor.matmul(
                                ps_o,
                                lhsT=wo_sb[:, k * 1024 + m * 128:
                                           k * 1024 + (m + 1) * 128],
                                rhs=catm[:, k * 256:(k + 1) * 256],
                                start=False, stop=(k == 7))
                        _selu_from_psum(nc, otmp, ps_o, cst_sb[:, 16 + m:17 + m],
                                        aoT[m][:, :], 128, 256,
                                        f"o_{m}", zeros=zeros)

        # ---- V/C logits stream (kvt/kvag/wo freed -> room for vct) ----
        with tc.tile_pool(name="vstream", bufs=24) as vsp, \
             tc.tile_pool(name="vstage", bufs=6) as vst, \
             tc.tile_pool(name="vps", bufs=2, space="PSUM") as vps:
            for g in range(NG):
                vtiles = []
                for k in range(8):
                    vt = vsp.tile([128, GW], BF16, name="vt", tag="vct")
                    nc.sync.dma_start(out=vt, in_=vctp[g, k, :, :])
                    vtiles.append(vt)
                for hh in range(2):
                    stg = vst.tile([128, GW], BF16, name="vstage",
                                   tag="vstage")
                    ps4 = vps.tile([128, 2048], F32, name="ps_v", tag="ps_v")
                    for u in range(4):
                        for k in range(8):
                            nc.tensor.matmul(
                                ps4[:, u * 512: u * 512 + VBW],
                                lhsT=aoT[k][:, hh * 128:(hh + 1) * 128],
                                rhs=vtiles[k][:, u * VBW:(u + 1) * VBW],
                                start=(k == 0), stop=(k == 7))
                    src = ps4.rearrange("p (u w) -> p u w", w=512)[:, :, 0:VBW]
                    dst = stg.rearrange("p (u w) -> p u w", w=VBW)
                    nc.vector.tensor_copy(out=dst, in_=src)
                    nc.gpsimd.dma_start(out=out[hh, g, :, :], in_=stg)


# ---------------- host side ----------------

def _wn_conv(v, g):
    n = np.sqrt((v * v).sum(axis=(1, 2), keepdims=True))
    return g[:, None, None] * v / n


def _wn_lin(v, g):
    return g[:, None] * v / np.linalg.norm(v, axis=1, keepdims=True)


def _selu_np(x):
    return np.where(x > 0, LAM * x,
                    LAM * ALPHA * (np.exp(np.minimum(x, 0)) - 1)).astype(np.float32)


def _bf16(x):
    return np.ascontiguousarray(x.astype(ml_dtypes.bfloat16))


def _f32(x):
    return np.ascontiguousarray(x.astype(np.float32))


_PROGRAM_CACHE = {}


def kernel(o, f, q0_v, q0_g, q0_b, q1_v, q1_g, q1_b,
           out_v, out_g, out_b, V_v, V_g, V_b, C_v, C_g, C_b):
    o, f = np.asarray(o), np.asarray(f)

    w0 = _wn_conv(np.asarray(q0_v), np.asarray(q0_g)) * LAM      # (H, CIN, 3)
    w1 = _wn_conv(np.asarray(q1_v), np.asarray(q1_g)) * LAM      # (H, H, 3)
    b0 = np.asarray(q0_b) * LAM
    b1 = np.asarray(q1_b) * LAM
    woutT = np.ascontiguousarray(_wn_lin(np.asarray(out_v), np.asarray(out_g)).T) * LAM
    outb_l = np.asarray(out_b) * LAM
    vc = np.concatenate([_wn_lin(np.asarray(V_v), np.asarray(V_g)),
                         _wn_lin(np.asarray(C_v), np.asarray(C_g))], axis=0)
    vct = np.ascontiguousarray(vc.T)                             # (H, 32512)
    kv = _selu_np(f)                                             # (S, H)

    # shared packed layouts
    w0T = w0.transpose(2, 1, 0).reshape(30, 128, H)              # (kc, 128, 1024)
    w0pk = _bf16(w0T.reshape(30, 128, 8, 128).transpose(1, 0, 2, 3)
                 .reshape(128, 30 * 1024))
    w1T = w1.transpose(2, 1, 0).reshape(24, 128, H)
    w1pk = _bf16(w1T.reshape(24, 128, 8, 128).transpose(1, 0, 2, 3)
                 .reshape(128, 24 * 1024))
    wopk = _bf16(woutT.reshape(16, 128, 1024).transpose(1, 0, 2)
                 .reshape(128, 16 * 1024))
    vctp = _bf16(vct.reshape(8, 128, NG, GW).transpose(2, 0, 1, 3))
    kvT_full = np.ascontiguousarray(kv.T)                        # (H, S)
    kvtpk = _bf16(kvT_full.reshape(8, 128, S).transpose(1, 0, 2)
                  .reshape(128, 8 * 2048))
    # kvag: [s, (wave-pair, st, hh, 65)]; col 64 of each 65-block = ones
    kvag = np.zeros((S, NH, 65), np.float32)
    for n in range(NH):
        kvag[:, n, 0:64] = kv[:, n * 64:(n + 1) * 64]
        kvag[:, n, 64] = 1.0
    # -> [128 s-part, w, st, hh, 65]
    kvagpk = _bf16(kvag.reshape(16, 128, 8, 2, 65).transpose(1, 2, 0, 3, 4)
                   .reshape(128, 8 * 2080))
    csti = np.zeros((128, 26), np.float32)
    csti[:, 0:8] = b0.reshape(8, 128).T
    csti[:, 8:16] = b1.reshape(8, 128).T
    csti[:, 16:24] = outb_l.reshape(8, 128).T

    # oT with 'same'-conv halo: core i needs o cols [256i-2, 256i+258)
    oT_pad = np.zeros((CIN, L + 4), np.float32)
    oT_pad[:, 2:L + 2] = o.T

    if "nc" not in _PROGRAM_CACHE:
        _PROGRAM_CACHE["nc"] = build_program()
    nc = _PROGRAM_CACHE["nc"]

    in_maps = []
    for i in range(NCORES):
        oTi = oT_pad[:, 256 * i:256 * i + 260]                   # (1280, 260)
        oTpi = _bf16(oTi.reshape(10, 128, 260).transpose(1, 0, 2)
                     .reshape(128, 10 * 260))
        ci = csti.copy()
        ci[:, 24] = 0.0 if i == 0 else 1.0
        ci[:, 25] = 0.0 if i == NCORES - 1 else 1.0
        in_maps.append({
            "oTp": oTpi,
            "w0p": w0pk,
            "w1p": w1pk,
            "kvtp": kvtpk,
            "kvagp": kvagpk,
            "wop": wopk,
            "cst": _f32(ci),
            "vctp": vctp,
        })

    kwargs = {}
    if os.environ.get("NN_COPY_TRACE", "0") == "1":
        kwargs = dict(trace=True)
    res = run_bass_kernel_spmd(nc, in_maps, core_ids=list(range(NCORES)), **kwargs)
    global LAST_RESULTS
    LAST_RESULTS = res
    # core i, local row lr = n*16 + pgl  ->  global row n*128 + 16i + pgl
    full = np.empty((NH, NCORES, 16, VC), np.float32)
    for i in range(NCORES):
        od = np.asarray(res.results[i]["out"]).astype(np.float32)  # (2,NG,128,GW)
        rows = od.transpose(0, 2, 1, 3).reshape(256, VC)           # (lr, VC)
        full[:, i, :, :] = rows.reshape(NH, 16, VC)
    full = full.reshape(L, VC)
    full += np.concatenate([np.asarray(V_b), np.asarray(C_b)])[None, :]
    return full


# revision 35
# speedup vs baseline: 1.2318x; 1.0039x over previous
"""Trainium2 Bass kernel for nn_Copy_56470230008202 (sparse_attention).

Strategy (8 NeuronCores, SPMD, one launch) -- collective-free L-sharding.

  The reference's `mixh.reshape(1,-1,H)` / `q2 = qh.transpose(1,0,2,3)`
  views scramble rows so that output row r = n*128 + pg (head n, position
  group pg) draws ONLY from q positions t = pg*16 + j (j=0..15) of head n.
  Hence a core that owns a contiguous 256-slice of L -- q positions
  [256i, 256i+256), i.e. pg in [16i, 16i+16) -- computes 256 COMPLETE
  output rows {r = n*128 + 16i + pgl} for ALL 16 heads with NO collective
  (vs v1's head-sharding, which needed an ~80us AllGather + warm-up window).

  - conv0/conv1 L-sharded: all 1024 channels over 258/256 local columns
    (halos computed redundantly; host pads oT; a per-core mask in cst
    zeroes the halo at the two global sequence edges, which are conv1's
    zero-pad positions).  w0 streams through a half-size ring buffer
    (phase A kc 0-14 kc-major while chunks land; phase B re-streams
    kc 15-29 into the same space, m-major so the selu evacuations
    pipeline with the matmul tail).  w1 loads in full into its own
    buffer during conv0 (aliasing w0 would serialize the load behind
    conv0's last matmul).  DMA rings split: w0-even/kv/wo on sync,
    w0-odd/w1 on the gpsimd SWDGE ring.
  - attention in 8 waves of (head pair, 256 local q-cols): scores N=256
    with the two heads at PE row-groups 0/64 (concurrent sub-arrays),
    exp on [128,1024] 2-bank psums (ACT is the phase pacer, saturated at
    ~76us), mix into per-head single-bank psums whose kvag lhsT carries
    64 REPLICATED ones columns -- the softmax denominator lands
    broadcast in psum partitions 64..127 (free: banks are per-partition)
    so the norm chain is pure DVE: partition-parallel reciprocal +
    strided scramble multiplies, fully hidden under the next score
    block.  Wave emission order: mix(w) | norm(w) | scores(w+1).
    CAUTION: one accumulation chain per psum bank -- start=True clears
    has_written for the WHOLE bank (measured; packing 2 heads in one
    bank silently drops the first head's early chunks).
  - out-proj with both heads merged in the free dim (N=256); V/C logits
    vs full vct streamed in 16 groups x 8 contiguous 520KB blocks,
    4-bank psum, DVE evacuation, SWDGE write-out (final stage split in
    4 to pipeline the tail).  This phase is PE-roofline (508 cols/MM at
    1 col/cycle; 214ns/MM at 2.4GHz) with zero stalls.
  All matmuls bf16 inputs / fp32 PSUM accumulation.  Weight-norm,
  selu(f), transposes, packing, sharding, final bias add on host.
  Measured: 413.7us HW exec (vs 616us v1 baseline), rel err 0.0058.
  NOTE: board-level clock management (GPIO 13/16 throttle / P0 ~2.0GHz)
  varies run-to-run and swings the total by +-45us at identical code.
"""

import os
import sys

for _p in ("/opt/trn_rl_repo", "/root/.axon_site/_ro/trn_rl_repo"):
    if os.path.isdir(_p) and _p not in sys.path:
        sys.path.append(_p)

import numpy as np
import ml_dtypes

import concourse.bass as bass
import concourse.mybir as mybir
from concourse import bacc
from concourse.tile import TileContext
from concourse.bass_utils import run_bass_kernel_spmd

F32 = mybir.dt.float32
BF16 = mybir.dt.bfloat16
ALU = mybir.AluOpType
ACTF = mybir.ActivationFunctionType

H, NH, HD = 1024, 16, 64
CIN, VOCAB, LIMIT, L, S = 1280, 32000, 512, 2048, 2048
VC = VOCAB + LIMIT              # 32512 = 16 groups * 2032 = 64 * 508
NVB, VBW = 64, 508
NG, GW = 16, 2032               # V-stream groups: 4 vocab blocks per group
NCORES = 8
LQ = L // NCORES                # 256 local q columns per core
LAM, ALPHA = 1.0507009873554805, 1.6732632423543772


def _selu_from_psum(nc, tmp, psum_ap, bias_ap, out_ap, P, N, idx, pbase=0,
                    zeros=None):
    """out = selu(z) given psum = LAM*z (lambda folded into weights+bias).
    selu(z) = max(y,0) + LAM*ALPHA*(exp(min(y,0)/LAM) - 1),  y = LAM*z + b'.
    """
    m = tmp.tile([P, N], F32, name=f"selu_m{idx}", tag=f"selu_m{P}x{N}")
    r = tmp.tile([P, N], F32, name=f"selu_r{idx}", tag=f"selu_r{P}x{N}")
    e = tmp.tile([P, N], F32, name=f"selu_e{idx}", tag=f"selu_e{P}x{N}")
    t = tmp.tile([P, N], F32, name=f"selu_t{idx}", tag=f"selu_t{P}x{N}")
    z = zeros[pbase:pbase + P, :N]
    nc.vector.scalar_tensor_tensor(m, psum_ap, bias_ap, z, op0=ALU.add, op1=ALU.min)
    nc.vector.scalar_tensor_tensor(r, psum_ap, bias_ap, z, op0=ALU.add, op1=ALU.max)
    nc.scalar.activation(e, m, ACTF.Exp, scale=1.0 / LAM)
    nc.vector.tensor_scalar(t, e, LAM * ALPHA, -LAM * ALPHA, op0=ALU.mult, op1=ALU.add)
    nc.vector.tensor_tensor(out_ap, t, r, op=ALU.add)


def build_program():
    nc = bacc.Bacc("TRN2", target_bir_lowering=False, debug=False,
                   num_devices=NCORES)
    # all inputs packed per-SBUF-tile contiguous (column blocks)
    oTp = nc.declare_dram_parameter("oTp", [128, 10 * 260], BF16, isOutput=False)
    w0p = nc.declare_dram_parameter("w0p", [128, 30 * 1024], BF16, isOutput=False)
    w1p = nc.declare_dram_parameter("w1p", [128, 24 * 1024], BF16, isOutput=False)
    kvtp = nc.declare_dram_parameter("kvtp", [128, 8 * 2048], BF16, isOutput=False)
    kvagp = nc.declare_dram_parameter("kvagp", [128, 8 * 4096], BF16, isOutput=False)
    wop = nc.declare_dram_parameter("wop", [128, 16 * 1024], BF16, isOutput=False)
    cst = nc.declare_dram_parameter("cst", [128, 26], F32, isOutput=False)
    vctp = nc.declare_dram_parameter("vctp", [NG, 8, 128, GW], BF16, isOutput=False)
    out = nc.declare_dram_parameter("out", [2, NG, 128, GW], BF16, isOutput=True)

    with TileContext(nc) as tc:
        _emit(tc, oTp, w0p, w1p, kvtp, kvagp, wop, cst, vctp, out)
    if not nc.is_finalized():
        nc.finalize()
    return nc


def _emit(tc, oTp, w0p, w1p, kvtp, kvagp, wop, cst, vctp, out):
    nc = tc.nc

    with tc.tile_pool(name="const", bufs=1) as constp, \
         tc.tile_pool(name="persist", bufs=1) as pers:
        zeros = constp.tile([128, 512], F32)
        nc.vector.memset(zeros, 0.0)
        cst_sb = constp.tile([128, 26], F32)

        # persistent activations
        q_sb = pers.tile([128, 8 * LQ], BF16)         # conv1 out, chunk m cols
        catm = pers.tile([128, 8 * LQ], BF16, name="catm")
        catq = pers.tile([128, 8 * LQ], BF16, name="catq")
        aoT = [pers.tile([128, LQ], BF16, name=f"aoT{m}") for m in range(8)]

        with tc.tile_pool(name="v0p", bufs=1) as v0p, \
             tc.tile_pool(name="kvp", bufs=1) as kvp:
            # vct group 0 prefetched during attention so the V/C stream
            # starts without a DMA wait (the main pool's space aliases kvt)
            v0tiles = [v0p.tile([128, GW], BF16, name=f"v0_{k}")
                       for k in range(8)]
            kvt_sb = kvp.tile([128, 8 * 2048], BF16)   # [hd ch, s] per pair
            kvag_sb = kvp.tile([128, 8 * 4096], BF16)  # [s,(st,hh,128)] per pair

            # ---------------- conv0 / conv1 ----------------
            with tc.tile_pool(name="c0", bufs=1) as c0p:
                oT_sb = c0p.tile([128, 10 * 260], BF16)
                x0 = c0p.tile([128, 8 * 258], BF16)
                w1_sb = c0p.tile([128, 24 * 1024], BF16)
                with tc.tile_pool(name="c0w", bufs=1) as c0w, \
                     tc.tile_pool(name="c0ps", bufs=1, space="PSUM") as c0ps, \
                     tc.tile_pool(name="c0tmp", bufs=3) as c0tmp:
                    # half-size ring: phase A holds kc 0..14, phase B
                    # re-streams kc 15..29 into the same addresses (the WAR
                    # deps against phase-A matmuls resolve chunk by chunk)
                    w0_sb = c0w.tile([128, 15 * 1024], BF16)
                    nc.sync.dma_start(out=oT_sb[:, 0:260], in_=oTp[:, 0:260])
                    nc.sync.dma_start(out=w0_sb[:, 0:1024], in_=w0p[:, 0:1024])
                    nc.sync.dma_start(out=cst_sb, in_=cst[:, :])
                    nc.sync.dma_start(out=w0_sb[:, 1024:3072],
                                      in_=w0p[:, 1024:3072])
                    nc.sync.dma_start(out=oT_sb[:, 260:2600],
                                      in_=oTp[:, 260:2600])
                    for j in range(1, 5):            # 3 kc-slices per DMA,
                        eng = nc.sync if j % 2 == 0 else nc.gpsimd
                        eng.dma_start(out=w0_sb[:, j * 3072:(j + 1) * 3072],
                                      in_=w0p[:, j * 3072:(j + 1) * 3072])
                    # full w1 (does NOT alias w0 -> streams during conv0)
                    for j in range(8):
                        nc.gpsimd.dma_start(
                            out=w1_sb[:, j * 3072:(j + 1) * 3072],
                            in_=w1p[:, j * 3072:(j + 1) * 3072])
                    pss = [c0ps.tile([128, 258], F32, name=f"c0ps{m}",
                                     tag=f"c0ps{m}") for m in range(8)]
                    # phase A: kc-major while weights stream in
                    for kc in range(15):
                        k, c = kc // 10, kc % 10
                        for m in range(8):
                            nc.tensor.matmul(
                                pss[m],
                                lhsT=w0_sb[:, (kc * 8 + m) * 128:
                                           (kc * 8 + m + 1) * 128],
                                rhs=oT_sb[:, c * 260 + k: c * 260 + k + 258],
                                start=(kc == 0), stop=False)
                    for j in range(5):               # phase-B re-stream
                        eng = nc.sync if j % 2 == 0 else nc.gpsimd
                        eng.dma_start(
                            out=w0_sb[:, j * 3072:(j + 1) * 3072],
                            in_=w0p[:, 15360 + j * 3072:15360 + (j + 1) * 3072])
                    # phase B: m-major so each psum finishes early and its
                    # selu evacuation overlaps the remaining matmuls
                    for m in range(8):
                        for kc in range(15, 30):
                            k, c = kc // 10, kc % 10
                            kb = kc - 15
                            nc.tensor.matmul(
                                pss[m],
                                lhsT=w0_sb[:, (kb * 8 + m) * 128:
                                           (kb * 8 + m + 1) * 128],
                                rhs=oT_sb[:, c * 260 + k: c * 260 + k + 258],
                                start=False, stop=(kc == 29))
                        _selu_from_psum(nc, c0tmp, pss[m], cst_sb[:, m:m + 1],
                                        x0[:, m * 258:(m + 1) * 258], 128, 258,
                                        f"c0_{m}", zeros=zeros)
                        # halo columns at the global sequence edges are conv1's
                        # zero-pad positions: mask (cores 0/7 get 0, else 1)
                        nc.vector.tensor_tensor(
                            out=x0[:, m * 258:m * 258 + 1],
                            in0=x0[:, m * 258:m * 258 + 1],
                            in1=cst_sb[:, 24:25], op=ALU.mult)
                        nc.vector.tensor_tensor(
                            out=x0[:, m * 258 + 257:m * 258 + 258],
                            in0=x0[:, m * 258 + 257:m * 258 + 258],
                            in1=cst_sb[:, 25:26], op=ALU.mult)

                with tc.tile_pool(name="c1ps", bufs=1, space="PSUM") as c1ps, \
                     tc.tile_pool(name="c1tmp", bufs=3) as c1tmp:
                    for w in range(8):
                        nc.sync.dma_start(out=kvt_sb[:, w * 2048:(w + 1) * 2048],
                                          in_=kvtp[:, w * 2048:(w + 1) * 2048])
                        nc.sync.dma_start(out=kvag_sb[:, w * 4096:(w + 1) * 4096],
                                          in_=kvagp[:, w * 4096:(w + 1) * 4096])
                    ps1 = [c1ps.tile([128, LQ], F32, name=f"c1ps{m}",
                                     tag=f"c1ps{m}") for m in range(8)]
                    for kc in range(12):
                        k, c = kc // 8, kc % 8
                        for m in range(8):
                            nc.tensor.matmul(
                                ps1[m],
                                lhsT=w1_sb[:, (kc * 8 + m) * 128:
                                           (kc * 8 + m + 1) * 128],
                                rhs=x0[:, c * 258 + k: c * 258 + k + 256],
                                start=(kc == 0), stop=False)
                    for m in range(8):
                        for kc in range(12, 24):
                            k, c = kc // 8, kc % 8
                            nc.tensor.matmul(
                                ps1[m],
                                lhsT=w1_sb[:, (kc * 8 + m) * 128:
                                           (kc * 8 + m + 1) * 128],
                                rhs=x0[:, c * 258 + k: c * 258 + k + 256],
                                start=False, stop=(kc == 23))
                        _selu_from_psum(nc, c1tmp, ps1[m], cst_sb[:, 8 + m:9 + m],
                                        q_sb[:, m * LQ:(m + 1) * LQ], 128, LQ,
                                        f"c1_{m}", zeros=zeros)

            # ------------- attention + scramble -------------
            with tc.tile_pool(name="wo", bufs=1) as wop_:
                wo_sb = wop_.tile([128, 16 * 1024], BF16)
                nc.sync.dma_start(out=wo_sb, in_=wop[:, :])
                for k in range(8):
                    nc.sync.dma_start(out=v0tiles[k], in_=vctp[0, k, :, :])

                with tc.tile_pool(name="ppool", bufs=8) as ppool, \
                     tc.tile_pool(name="rbcp", bufs=2) as rbcp, \
                     tc.tile_pool(name="scps", bufs=2, space="PSUM") as scps, \
                     tc.tile_pool(name="mixps", bufs=4, space="PSUM") as mixps:
                    # catq scramble: catq[jlo*64+d, kk*256 + n*16 + pgl]
                    #   = q_sb[m=n//2][(n%2)*64+d, pgl*16 + kk*2 + jlo]
                    cqre = catq.rearrange("p (kk c) -> p kk c", c=LQ)
                    for n in range(NH):
                        m, hh = n // 2, n % 2
                        qre = q_sb[:, m * LQ:(m + 1) * LQ].rearrange(
                            "p (pgl kk jlo) -> p jlo kk pgl", kk=8, jlo=2)
                        for jlo in range(2):
                            nc.vector.tensor_copy(
                                out=cqre[jlo * 64:(jlo + 1) * 64, :,
                                         n * 16:(n + 1) * 16],
                                in_=qre[hh * 64:(hh + 1) * 64, jlo, :, :])

                    def sc_psum(w, j):
                        # 4 score tiles (hh, st parity) in one 2-bank psum,
                        # heads at PE row-groups 0/64 run concurrently
                        ps = scps.tile([128, 1024], F32, name="ps_sc",
                                       tag="ps_sc")
                        for hh in range(2):
                            for par in range(2):
                                st = 2 * j + par
                                nc.tensor.matmul(
                                    ps[:, hh * 512 + par * 256:
                                       hh * 512 + (par + 1) * 256],
                                    lhsT=kvt_sb[hh * 64:(hh + 1) * 64,
                                                w * 2048 + st * 128:
                                                w * 2048 + (st + 1) * 128],
                                    rhs=q_sb[hh * 64:(hh + 1) * 64,
                                             w * LQ:(w + 1) * LQ],
                                    start=True, stop=True)
                        p = ppool.tile([128, 1024], BF16, name="p_t", tag="p")
                        nc.scalar.activation(p, ps, ACTF.Exp, scale=0.125)
                        return p

                    def mix_pair(w, j, plist, pms):
                        # accumulate st = 2j, 2j+1 for both heads.  kvag rows
                        # 64..127 are ones, so psum partitions 64..127 get the
                        # softmax denominator broadcast for free (the bank's
                        # upper partitions were unused anyway).  NOTE: the two
                        # heads need SEPARATE psum banks -- a start=True
                        # matmul clears has_written for its whole bank.
                        for hh in range(2):
                            for par in range(2):
                                st = 2 * j + par
                                nc.tensor.matmul(
                                    pms[hh][:, 0:256],
                                    lhsT=kvag_sb[:,
                                                 w * 4096 + st * 256 + hh * 128:
                                                 w * 4096 + st * 256 + (hh + 1) * 128],
                                    rhs=plist[j][:, hh * 512 + par * 256:
                                                 hh * 512 + (par + 1) * 256],
                                    start=(st == 0), stop=(st == 15))

                    def emit_norm(w, pms):
                        # pure-DVE chain: partition-parallel reciprocal of the
                        # broadcast denominator (psum rows 64..127), then the
                        # scramble multiplies.  No PE involvement, so it runs
                        # entirely under the next score block.
                        cmre = catm.rearrange("p (kk c) -> p kk c", c=LQ)
                        for hh in range(2):
                            n = 2 * w + hh
                            rbc = rbcp.tile([64, 256], F32, name="rbc",
                                            tag=f"rbc{hh}")
                            nc.vector.reciprocal(rbc, pms[hh][64:128, 0:256])
                            mre = pms[hh][0:64, 0:256].rearrange(
                                "p (pgl kk jlo) -> p jlo kk pgl", kk=8, jlo=2)
                            rre = rbc.rearrange(
                                "p (pgl kk jlo) -> p jlo kk pgl", kk=8, jlo=2)
                            for jlo in range(2):
                                nc.vector.tensor_tensor(
                                    out=cmre[jlo * 64:(jlo + 1) * 64, :,
                                             n * 16:(n + 1) * 16],
                                    in0=mre[:, jlo, :, :],
                                    in1=rre[:, jlo, :, :],
                                    op=ALU.mult)

                    # software pipeline, block order per wave w:
                    # mix(w) | scores(w+1) | norm(w) -- the score block sits
                    # between the last mix and the next wave's first mix, so
                    # the norm chain (dnm/bc/recip/scramble) completes during
                    # it instead of stalling mix(w+1) on the psum slots.
                    plist = [sc_psum(0, j) for j in range(8)]
                    for w in range(8):
                        pms = [mixps.tile([128, 512], F32, name="ps_mix",
                                          tag="ps_mix") for _ in range(2)]
                        for j in range(8):
                            mix_pair(w, j, plist, pms)
                        emit_norm(w, pms)
                        nxt = []
                        if w + 1 < 8:
                            for j in range(8):
                                nxt.append(sc_psum(w + 1, j))
                        plist = nxt

                # ---- out-projection (wo still resident) ----
                with tc.tile_pool(name="otmp", bufs=2) as otmp, \
                     tc.tile_pool(name="ops", bufs=2, space="PSUM") as ops:
                    for m in range(8):
                        ps_o = ops.tile([128, 256], F32, name="ps_o",
                                        tag="ps_o")
                        for k in range(8):
                            nc.tensor.matmul(
                                ps_o,
                                lhsT=wo_sb[:, (8 + k) * 1024 + m * 128:
                                           (8 + k) * 1024 + (m + 1) * 128],
                                rhs=catq[:, k * 256:(k + 1) * 256],
                                start=(k == 0), stop=False)
                        for k in range(8):
                            nc.tensor.matmul(
                                ps_o,
                                lhsT=wo_sb[:, k * 1024 + m * 128:
                                           k * 1024 + (m + 1) * 128],
                                rhs=catm[:, k * 256:(k + 1) * 256],
                                start=False, stop=(k == 7))
                        _selu_from_psum(nc, otmp, ps_o, cst_sb[:, 16 + m:17 + m],
                                        aoT[m][:, :], 128, 256,
                                        f"o_{m}", zeros=zeros)

        # ---- V/C logits stream (kvt/kvag/wo freed -> room for vct) ----
        with tc.tile_pool(name="vstream", bufs=24) as vsp, \
             tc.tile_pool(name="vstage", bufs=6) as vst, \
             tc.tile_pool(name="vps", bufs=2, space="PSUM") as vps:
            for g in range(NG):
                if g == 0:
                    vtiles = v0tiles
                else:
                    vtiles = []
                    for k in range(8):
                        vt = vsp.tile([128, GW], BF16, name="vt", tag="vct")
                        nc.sync.dma_start(out=vt, in_=vctp[g, k, :, :])
                        vtiles.append(vt)
                for hh in range(2):
                    stg = vst.tile([128, GW], BF16, name="vstage",
                                   tag="vstage")
                    ps4 = vps.tile([128, 2048], F32, name="ps_v", tag="ps_v")
                    for u in range(4):
                        for k in range(8):
                            nc.tensor.matmul(
                                ps4[:, u * 512: u * 512 + VBW],
                                lhsT=aoT[k][:, hh * 128:(hh + 1) * 128],
                                rhs=vtiles[k][:, u * VBW:(u + 1) * VBW],
                                start=(k == 0), stop=(k == 7))
                    src = ps4.rearrange("p (u w) -> p u w", w=512)[:, :, 0:VBW]
                    dst = stg.rearrange("p (u w) -> p u w", w=VBW)
                    last = (g == NG - 1 and hh == 1)
                    if not last:
                        nc.vector.tensor_copy(out=dst, in_=src)
                        nc.gpsimd.dma_start(out=out[hh, g, :, :], in_=stg)
                    else:
                        # split the final stage so its evacuation + write-out
                        # pipeline instead of serializing after the last MM
                        for u in range(4):
                            nc.vector.tensor_copy(
                                out=dst[:, u, :], in_=src[:, u, :])
                            nc.sync.dma_start(
                                out=out[hh, g, :, u * VBW:(u + 1) * VBW],
                                in_=stg[:, u * VBW:(u + 1) * VBW])


# ---------------- host side ----------------

def _wn_conv(v, g):
    n = np.sqrt((v * v).sum(axis=(1, 2), keepdims=True))
    return g[:, None, None] * v / n


def _wn_lin(v, g):
    return g[:, None] * v / np.linalg.norm(v, axis=1, keepdims=True)


def _selu_np(x):
    return np.where(x > 0, LAM * x,
                    LAM * ALPHA * (np.exp(np.minimum(x, 0)) - 1)).astype(np.float32)


def _bf16(x):
    return np.ascontiguousarray(x.astype(ml_dtypes.bfloat16))


def _f32(x):
    return np.ascontiguousarray(x.astype(np.float32))


_PROGRAM_CACHE = {}


def kernel(o, f, q0_v, q0_g, q0_b, q1_v, q1_g, q1_b,
           out_v, out_g, out_b, V_v, V_g, V_b, C_v, C_g, C_b):
    o, f = np.asarray(o), np.asarray(f)

    w0 = _wn_conv(np.asarray(q0_v), np.asarray(q0_g)) * LAM      # (H, CIN, 3)
    w1 = _wn_conv(np.asarray(q1_v), np.asarray(q1_g)) * LAM      # (H, H, 3)
    b0 = np.asarray(q0_b) * LAM
    b1 = np.asarray(q1_b) * LAM
    woutT = np.ascontiguousarray(_wn_lin(np.asarray(out_v), np.asarray(out_g)).T) * LAM
    outb_l = np.asarray(out_b) * LAM
    vc = np.concatenate([_wn_lin(np.asarray(V_v), np.asarray(V_g)),
                         _wn_lin(np.asarray(C_v), np.asarray(C_g))], axis=0)
    vct = np.ascontiguousarray(vc.T)                             # (H, 32512)
    kv = _selu_np(f)                                             # (S, H)

    # shared packed layouts
    w0T = w0.transpose(2, 1, 0).reshape(30, 128, H)              # (kc, 128, 1024)
    w0pk = _bf16(w0T.reshape(30, 128, 8, 128).transpose(1, 0, 2, 3)
                 .reshape(128, 30 * 1024))
    w1T = w1.transpose(2, 1, 0).reshape(24, 128, H)
    w1pk = _bf16(w1T.reshape(24, 128, 8, 128).transpose(1, 0, 2, 3)
                 .reshape(128, 24 * 1024))
    wopk = _bf16(woutT.reshape(16, 128, 1024).transpose(1, 0, 2)
                 .reshape(128, 16 * 1024))
    vctp = _bf16(vct.reshape(8, 128, NG, GW).transpose(2, 0, 1, 3))
    kvT_full = np.ascontiguousarray(kv.T)                        # (H, S)
    kvtpk = _bf16(kvT_full.reshape(8, 128, S).transpose(1, 0, 2)
                  .reshape(128, 8 * 2048))
    # kvag: [s, (wave-pair, st, hh, 128)]; cols 64..127 = ones, so the
    # mix matmul broadcasts the softmax denominator into psum rows 64..127
    kvag = np.zeros((S, NH, 128), np.float32)
    for n in range(NH):
        kvag[:, n, 0:64] = kv[:, n * 64:(n + 1) * 64]
        kvag[:, n, 64:128] = 1.0
    kvagpk = _bf16(kvag.reshape(16, 128, 8, 2, 128).transpose(1, 2, 0, 3, 4)
                   .reshape(128, 8 * 4096))
    csti = np.zeros((128, 26), np.float32)
    csti[:, 0:8] = b0.reshape(8, 128).T
    csti[:, 8:16] = b1.reshape(8, 128).T
    csti[:, 16:24] = outb_l.reshape(8, 128).T

    # oT with 'same'-conv halo: core i needs o cols [256i-2, 256i+258)
    oT_pad = np.zeros((CIN, L + 4), np.float32)
    oT_pad[:, 2:L + 2] = o.T

    if "nc" not in _PROGRAM_CACHE:
        _PROGRAM_CACHE["nc"] = build_program()
    nc = _PROGRAM_CACHE["nc"]

    in_maps = []
    for i in range(NCORES):
        oTi = oT_pad[:, 256 * i:256 * i + 260]                   # (1280, 260)
        oTpi = _bf16(oTi.reshape(10, 128, 260).transpose(1, 0, 2)
                     .reshape(128, 10 * 260))
        ci = csti.copy()
        ci[:, 24] = 0.0 if i == 0 else 1.0
        ci[:, 25] = 0.0 if i == NCORES - 1 else 1.0
        in_maps.append({
            "oTp": oTpi,
            "w0p": w0pk,
            "w1p": w1pk,
            "kvtp": kvtpk,
            "kvagp": kvagpk,
            "wop": wopk,
            "cst": _f32(ci),
            "vctp": vctp,
        })

    kwargs = {}
    if os.environ.get("NN_COPY_TRACE", "0") == "1":
        kwargs = dict(trace=True)
    res = run_bass_kernel_spmd(nc, in_maps, core_ids=list(range(NCORES)), **kwargs)
    global LAST_RESULTS
    LAST_RESULTS = res
    # core i, local row lr = n*16 + pgl  ->  global row n*128 + 16i + pgl
    full = np.empty((NH, NCORES, 16, VC), np.float32)
    for i in range(NCORES):
        od = np.asarray(res.results[i]["out"]).astype(np.float32)  # (2,NG,128,GW)
        rows = od.transpose(0, 2, 1, 3).reshape(256, VC)           # (lr, VC)
        full[:, i, :, :] = rows.reshape(NH, 16, VC)
    full = full.reshape(L, VC)
    full += np.concatenate([np.asarray(V_b), np.asarray(C_b)])[None, :]
    return full


# revision 36
# speedup vs baseline: 1.2815x; 1.0404x over previous
"""Trainium2 Bass kernel for nn_Copy_56470230008202 (sparse_attention).

Strategy (8 NeuronCores, SPMD, one launch) -- collective-free L-sharding.

  The reference's `mixh.reshape(1,-1,H)` / `q2 = qh.transpose(1,0,2,3)`
  views scramble rows so that output row r = n*128 + pg (head n, position
  group pg) draws ONLY from q positions t = pg*16 + j (j=0..15) of head n.
  Hence a core that owns a contiguous 256-slice of L -- q positions
  [256i, 256i+256), i.e. pg in [16i, 16i+16) -- computes 256 COMPLETE
  output rows {r = n*128 + 16i + pgl} for ALL 16 heads with NO collective
  (vs v1's head-sharding, which needed an ~80us AllGather + warm-up window).

  - conv0/conv1 L-sharded: all 1024 channels over 258/256 local columns
    (halos computed redundantly; host pads oT; a per-core mask in cst
    zeroes the halo at the two global sequence edges, which are conv1's
    zero-pad positions).  w0 streams through a half-size ring buffer
    (phase A kc 0-14 kc-major while chunks land; phase B re-streams
    kc 15-29 into the same space, m-major so the selu evacuations
    pipeline with the matmul tail).  w1 loads in full into its own
    buffer during conv0 (aliasing w0 would serialize the load behind
    conv0's last matmul).  DMA rings split: w0-even/kv/wo on sync,
    w0-odd/w1 on the gpsimd SWDGE ring.
  - attention in 8 waves of (head pair, 256 local q-cols): scores N=256
    with the two heads at PE row-groups 0/64 (concurrent sub-arrays),
    exp on [128,1024] 2-bank psums (ACT is the phase pacer, saturated at
    ~76us), mix into per-head single-bank psums whose kvag lhsT carries
    64 REPLICATED ones columns -- the softmax denominator lands
    broadcast in psum partitions 64..127 (free: banks are per-partition)
    so the norm chain is pure DVE: partition-parallel reciprocal +
    strided scramble multiplies, fully hidden under the next score
    block.  Wave emission order: mix(w) | norm(w) | scores(w+1).
    CAUTION: one accumulation chain per psum bank -- start=True clears
    has_written for the WHOLE bank (measured; packing 2 heads in one
    bank silently drops the first head's early chunks).
  - out-proj with both heads merged in the free dim (N=256); V/C logits
    vs full vct streamed in 16 groups x 8 contiguous 520KB blocks,
    4-bank psum, DVE evacuation, SWDGE write-out (final stage split in
    4 to pipeline the tail).  This phase is PE-roofline (508 cols/MM at
    1 col/cycle; 214ns/MM at 2.4GHz) with zero stalls.
  All matmuls bf16 inputs / fp32 PSUM accumulation.  Weight-norm,
  selu(f), transposes, packing, sharding, final bias add on host.
  Measured: 413.7us HW exec (vs 616us v1 baseline), rel err 0.0058.
  NOTE: board-level clock management (GPIO 13/16 throttle / P0 ~2.0GHz)
  varies run-to-run and swings the total by +-45us at identical code.
"""

import os
import sys

for _p in ("/opt/trn_rl_repo", "/root/.axon_site/_ro/trn_rl_repo"):
    if os.path.isdir(_p) and _p not in sys.path:
        sys.path.append(_p)

import numpy as np
import ml_dtypes

import concourse.bass as bass
import concourse.mybir as mybir
from concourse import bacc
from concourse.tile import TileContext
from concourse.bass_utils import run_bass_kernel_spmd

F32 = mybir.dt.float32
BF16 = mybir.dt.bfloat16
ALU = mybir.AluOpType
ACTF = mybir.ActivationFunctionType

H, NH, HD = 1024, 16, 64
CIN, VOCAB, LIMIT, L, S = 1280, 32000, 512, 2048, 2048
VC = VOCAB + LIMIT              # 32512 = 16 groups * 2032 = 64 * 508
NVB, VBW = 64, 508
NG, GW = 16, 2032               # V-stream groups: 4 vocab blocks per group
NCORES = 8
LQ = L // NCORES                # 256 local q columns per core
LAM, ALPHA = 1.0507009873554805, 1.6732632423543772


def _selu_from_psum(nc, tmp, psum_ap, bias_ap, out_ap, P, N, idx, pbase=0,
                    zeros=None):
    """out = selu(z) given psum = LAM*z (lambda folded into weights+bias).
    selu(z) = max(y,0) + LAM*ALPHA*(exp(min(y,0)/LAM) - 1),  y = LAM*z + b'.
    """
    m = tmp.tile([P, N], F32, name=f"selu_m{idx}", tag=f"selu_m{P}x{N}")
    r = tmp.tile([P, N], F32, name=f"selu_r{idx}", tag=f"selu_r{P}x{N}")
    e = tmp.tile([P, N], F32, name=f"selu_e{idx}", tag=f"selu_e{P}x{N}")
    t = tmp.tile([P, N], F32, name=f"selu_t{idx}", tag=f"selu_t{P}x{N}")
    z = zeros[pbase:pbase + P, :N]
    nc.vector.scalar_tensor_tensor(m, psum_ap, bias_ap, z, op0=ALU.add, op1=ALU.min)
    nc.vector.scalar_tensor_tensor(r, psum_ap, bias_ap, z, op0=ALU.add, op1=ALU.max)
    nc.scalar.activation(e, m, ACTF.Exp, scale=1.0 / LAM)
    nc.vector.tensor_scalar(t, e, LAM * ALPHA, -LAM * ALPHA, op0=ALU.mult, op1=ALU.add)
    nc.vector.tensor_tensor(out_ap, t, r, op=ALU.add)


def build_program():
    nc = bacc.Bacc("TRN2", target_bir_lowering=False, debug=False,
                   num_devices=NCORES)
    # all inputs packed per-SBUF-tile contiguous (column blocks)
    oTp = nc.declare_dram_parameter("oTp", [128, 10 * 260], BF16, isOutput=False)
    w0p = nc.declare_dram_parameter("w0p", [128, 30 * 1024], BF16, isOutput=False)
    w1p = nc.declare_dram_parameter("w1p", [128, 24 * 1024], BF16, isOutput=False)
    kvtp = nc.declare_dram_parameter("kvtp", [128, 8 * 2048], BF16, isOutput=False)
    kvagp = nc.declare_dram_parameter("kvagp", [128, 8 * 4096], BF16, isOutput=False)
    wop = nc.declare_dram_parameter("wop", [128, 16 * 1024], BF16, isOutput=False)
    cst = nc.declare_dram_parameter("cst", [128, 26], F32, isOutput=False)
    vctp = nc.declare_dram_parameter("vctp", [NG, 8, 128, GW], BF16, isOutput=False)
    out = nc.declare_dram_parameter("out", [2, NG, 128, GW], BF16, isOutput=True)

    with TileContext(nc) as tc:
        _emit(tc, oTp, w0p, w1p, kvtp, kvagp, wop, cst, vctp, out)
    if not nc.is_finalized():
        nc.finalize()
    return nc


def _emit(tc, oTp, w0p, w1p, kvtp, kvagp, wop, cst, vctp, out):
    nc = tc.nc

    with tc.tile_pool(name="const", bufs=1) as constp, \
         tc.tile_pool(name="persist", bufs=1) as pers:
        zeros = constp.tile([128, 512], F32)
        nc.vector.memset(zeros, 0.0)
        cst_sb = constp.tile([128, 26], F32)

        # persistent activations
        q_sb = pers.tile([128, 8 * LQ], BF16)         # conv1 out, chunk m cols
        catm = pers.tile([128, 8 * LQ], BF16, name="catm")
        catq = pers.tile([128, 8 * LQ], BF16, name="catq")
        aoT = [pers.tile([128, LQ], BF16, name=f"aoT{m}") for m in range(8)]

        with tc.tile_pool(name="v0p", bufs=1) as v0p, \
             tc.tile_pool(name="kvp", bufs=1) as kvp:
            # vct group 0 prefetched during attention so the V/C stream
            # starts without a DMA wait (the main pool's space aliases kvt)
            v0tiles = [v0p.tile([128, GW], BF16, name=f"v0_{k}")
                       for k in range(8)]
            kvt_sb = kvp.tile([128, 8 * 2048], BF16)   # [hd ch, s] per pair
            kvag_sb = kvp.tile([128, 8 * 4096], BF16)  # [s,(st,hh,128)] per pair

            # ---------------- conv0 / conv1 ----------------
            with tc.tile_pool(name="c0", bufs=1) as c0p:
                oT_sb = c0p.tile([128, 10 * 260], BF16)
                x0 = c0p.tile([128, 8 * 258], BF16)
                w1_sb = c0p.tile([128, 24 * 1024], BF16)
                with tc.tile_pool(name="c0w", bufs=1) as c0w, \
                     tc.tile_pool(name="c0ps", bufs=1, space="PSUM") as c0ps, \
                     tc.tile_pool(name="c0tmp", bufs=3) as c0tmp:
                    # half-size ring: phase A holds kc 0..14, phase B
                    # re-streams kc 15..29 into the same addresses (the WAR
                    # deps against phase-A matmuls resolve chunk by chunk)
                    w0_sb = c0w.tile([128, 15 * 1024], BF16)
                    nc.sync.dma_start(out=oT_sb[:, 0:260], in_=oTp[:, 0:260])
                    nc.sync.dma_start(out=w0_sb[:, 0:1024], in_=w0p[:, 0:1024])
                    nc.sync.dma_start(out=cst_sb, in_=cst[:, :])
                    nc.sync.dma_start(out=w0_sb[:, 1024:3072],
                                      in_=w0p[:, 1024:3072])
                    nc.sync.dma_start(out=oT_sb[:, 260:2600],
                                      in_=oTp[:, 260:2600])
                    for j in range(1, 5):            # 3 kc-slices per DMA,
                        eng = nc.sync if j % 2 == 0 else nc.gpsimd
                        eng.dma_start(out=w0_sb[:, j * 3072:(j + 1) * 3072],
                                      in_=w0p[:, j * 3072:(j + 1) * 3072])
                    # full w1 (does NOT alias w0 -> streams during conv0)
                    for j in range(8):
                        nc.gpsimd.dma_start(
                            out=w1_sb[:, j * 3072:(j + 1) * 3072],
                            in_=w1p[:, j * 3072:(j + 1) * 3072])
                    pss = [c0ps.tile([128, 258], F32, name=f"c0ps{m}",
                                     tag=f"c0ps{m}") for m in range(8)]
                    # phase A: kc-major while weights stream in
                    for kc in range(15):
                        k, c = kc // 10, kc % 10
                        for m in range(8):
                            nc.tensor.matmul(
                                pss[m],
                                lhsT=w0_sb[:, (kc * 8 + m) * 128:
                                           (kc * 8 + m + 1) * 128],
                                rhs=oT_sb[:, c * 260 + k: c * 260 + k + 258],
                                start=(kc == 0), stop=False)
                    for j in range(5):               # phase-B re-stream
                        eng = nc.sync if j % 2 == 0 else nc.gpsimd
                        eng.dma_start(
                            out=w0_sb[:, j * 3072:(j + 1) * 3072],
                            in_=w0p[:, 15360 + j * 3072:15360 + (j + 1) * 3072])
                    # phase B: m-major so each psum finishes early and its
                    # selu evacuation overlaps the remaining matmuls
                    for m in range(8):
                        for kc in range(15, 30):
                            k, c = kc // 10, kc % 10
                            kb = kc - 15
                            nc.tensor.matmul(
                                pss[m],
                                lhsT=w0_sb[:, (kb * 8 + m) * 128:
                                           (kb * 8 + m + 1) * 128],
                                rhs=oT_sb[:, c * 260 + k: c * 260 + k + 258],
                                start=False, stop=(kc == 29))
                        _selu_from_psum(nc, c0tmp, pss[m], cst_sb[:, m:m + 1],
                                        x0[:, m * 258:(m + 1) * 258], 128, 258,
                                        f"c0_{m}", zeros=zeros)
                        # halo columns at the global sequence edges are conv1's
                        # zero-pad positions: mask (cores 0/7 get 0, else 1)
                        nc.vector.tensor_tensor(
                            out=x0[:, m * 258:m * 258 + 1],
                            in0=x0[:, m * 258:m * 258 + 1],
                            in1=cst_sb[:, 24:25], op=ALU.mult)
                        nc.vector.tensor_tensor(
                            out=x0[:, m * 258 + 257:m * 258 + 258],
                            in0=x0[:, m * 258 + 257:m * 258 + 258],
                            in1=cst_sb[:, 25:26], op=ALU.mult)

                with tc.tile_pool(name="c1ps", bufs=1, space="PSUM") as c1ps, \
                     tc.tile_pool(name="c1tmp", bufs=3) as c1tmp:
                    for w in range(8):
                        nc.sync.dma_start(out=kvt_sb[:, w * 2048:(w + 1) * 2048],
                                          in_=kvtp[:, w * 2048:(w + 1) * 2048])
                        nc.sync.dma_start(out=kvag_sb[:, w * 4096:(w + 1) * 4096],
                                          in_=kvagp[:, w * 4096:(w + 1) * 4096])
                    ps1 = [c1ps.tile([128, LQ], F32, name=f"c1ps{m}",
                                     tag=f"c1ps{m}") for m in range(8)]
                    for kc in range(12):
                        k, c = kc // 8, kc % 8
                        for m in range(8):
                            nc.tensor.matmul(
                                ps1[m],
                                lhsT=w1_sb[:, (kc * 8 + m) * 128:
                                           (kc * 8 + m + 1) * 128],
                                rhs=x0[:, c * 258 + k: c * 258 + k + 256],
                                start=(kc == 0), stop=False)
                    for m in range(8):
                        for kc in range(12, 24):
                            k, c = kc // 8, kc % 8
                            nc.tensor.matmul(
                                ps1[m],
                                lhsT=w1_sb[:, (kc * 8 + m) * 128:
                                           (kc * 8 + m + 1) * 128],
                                rhs=x0[:, c * 258 + k: c * 258 + k + 256],
                                start=False, stop=(kc == 23))
                        _selu_from_psum(nc, c1tmp, ps1[m], cst_sb[:, 8 + m:9 + m],
                                        q_sb[:, m * LQ:(m + 1) * LQ], 128, LQ,
                                        f"c1_{m}", zeros=zeros)

            # ------------- attention + scramble -------------
            with tc.tile_pool(name="wo", bufs=1) as wop_:
                wo_sb = wop_.tile([128, 16 * 1024], BF16)
                nc.sync.dma_start(out=wo_sb, in_=wop[:, :])
                for k in range(8):
                    nc.sync.dma_start(out=v0tiles[k], in_=vctp[0, k, :, :])

                with tc.tile_pool(name="ppool", bufs=8) as ppool, \
                     tc.tile_pool(name="rbcp", bufs=2) as rbcp, \
                     tc.tile_pool(name="scps", bufs=2, space="PSUM") as scps, \
                     tc.tile_pool(name="mixps", bufs=4, space="PSUM") as mixps:
                    # catq scramble: catq[jlo*64+d, kk*256 + n*16 + pgl]
                    #   = q_sb[m=n//2][(n%2)*64+d, pgl*16 + kk*2 + jlo]
                    cqre = catq.rearrange("p (kk c) -> p kk c", c=LQ)
                    for n in range(NH):
                        m, hh = n // 2, n % 2
                        qre = q_sb[:, m * LQ:(m + 1) * LQ].rearrange(
                            "p (pgl kk jlo) -> p jlo kk pgl", kk=8, jlo=2)
                        for jlo in range(2):
                            nc.vector.tensor_copy(
                                out=cqre[jlo * 64:(jlo + 1) * 64, :,
                                         n * 16:(n + 1) * 16],
                                in_=qre[hh * 64:(hh + 1) * 64, jlo, :, :])

                    def sc_psum(w, j):
                        # 4 score tiles (hh, st parity) in one 2-bank psum,
                        # heads at PE row-groups 0/64 run concurrently
                        ps = scps.tile([128, 1024], F32, name="ps_sc",
                                       tag="ps_sc")
                        for hh in range(2):
                            for par in range(2):
                                st = 2 * j + par
                                nc.tensor.matmul(
                                    ps[:, hh * 512 + par * 256:
                                       hh * 512 + (par + 1) * 256],
                                    lhsT=kvt_sb[hh * 64:(hh + 1) * 64,
                                                w * 2048 + st * 128:
                                                w * 2048 + (st + 1) * 128],
                                    rhs=q_sb[hh * 64:(hh + 1) * 64,
                                             w * LQ:(w + 1) * LQ],
                                    start=True, stop=True)
                        p = ppool.tile([128, 1024], BF16, name="p_t", tag="p")
                        nc.scalar.activation(p, ps, ACTF.Exp, scale=0.125)
                        return p

                    def mix_pair(w, j, plist, pms):
                        # accumulate st = 2j, 2j+1 for both heads.  kvag rows
                        # 64..127 are ones, so psum partitions 64..127 get the
                        # softmax denominator broadcast for free (the bank's
                        # upper partitions were unused anyway).  NOTE: the two
                        # heads need SEPARATE psum banks -- a start=True
                        # matmul clears has_written for its whole bank.
                        for hh in range(2):
                            for par in range(2):
                                st = 2 * j + par
                                nc.tensor.matmul(
                                    pms[hh][:, 0:256],
                                    lhsT=kvag_sb[:,
                                                 w * 4096 + st * 256 + hh * 128:
                                                 w * 4096 + st * 256 + (hh + 1) * 128],
                                    rhs=plist[j][:, hh * 512 + par * 256:
                                                 hh * 512 + (par + 1) * 256],
                                    start=(st == 0), stop=(st == 15))

                    def emit_norm(w, pms):
                        # pure-DVE chain: partition-parallel reciprocal of the
                        # broadcast denominator (psum rows 64..127), then the
                        # scramble multiplies.  No PE involvement, so it runs
                        # entirely under the next score block.
                        cmre = catm.rearrange("p (kk c) -> p kk c", c=LQ)
                        for hh in range(2):
                            n = 2 * w + hh
                            rbc = rbcp.tile([64, 256], F32, name="rbc",
                                            tag=f"rbc{hh}")
                            nc.vector.reciprocal(rbc, pms[hh][64:128, 0:256])
                            mre = pms[hh][0:64, 0:256].rearrange(
                                "p (pgl kk jlo) -> p jlo kk pgl", kk=8, jlo=2)
                            rre = rbc.rearrange(
                                "p (pgl kk jlo) -> p jlo kk pgl", kk=8, jlo=2)
                            for jlo in range(2):
                                nc.vector.tensor_tensor(
                                    out=cmre[jlo * 64:(jlo + 1) * 64, :,
                                             n * 16:(n + 1) * 16],
                                    in0=mre[:, jlo, :, :],
                                    in1=rre[:, jlo, :, :],
                                    op=ALU.mult)

                    # software pipeline, block order per wave w:
                    # mix(w) | scores(w+1) | norm(w) -- the score block sits
                    # between the last mix and the next wave's first mix, so
                    # the norm chain (dnm/bc/recip/scramble) completes during
                    # it instead of stalling mix(w+1) on the psum slots.
                    plist = [sc_psum(0, j) for j in range(8)]
                    for w in range(8):
                        pms = [mixps.tile([128, 512], F32, name="ps_mix",
                                          tag="ps_mix") for _ in range(2)]
                        for j in range(8):
                            mix_pair(w, j, plist, pms)
                        emit_norm(w, pms)
                        nxt = []
                        if w + 1 < 8:
                            for j in range(8):
                                nxt.append(sc_psum(w + 1, j))
                        plist = nxt

                # ---- out-projection (wo still resident) ----
                with tc.tile_pool(name="otmp", bufs=2) as otmp, \
                     tc.tile_pool(name="ops", bufs=2, space="PSUM") as ops:
                    for m in range(8):
                        ps_o = ops.tile([128, 256], F32, name="ps_o",
                                        tag="ps_o")
                        for k in range(8):
                            nc.tensor.matmul(
                                ps_o,
                                lhsT=wo_sb[:, (8 + k) * 1024 + m * 128:
                                           (8 + k) * 1024 + (m + 1) * 128],
                                rhs=catq[:, k * 256:(k + 1) * 256],
                                start=(k == 0), stop=False)
                        for k in range(8):
                            nc.tensor.matmul(
                                ps_o,
                                lhsT=wo_sb[:, k * 1024 + m * 128:
                                           k * 1024 + (m + 1) * 128],
                                rhs=catm[:, k * 256:(k + 1) * 256],
                                start=False, stop=(k == 7))
                        _selu_from_psum(nc, otmp, ps_o, cst_sb[:, 16 + m:17 + m],
                                        aoT[m][:, :], 128, 256,
                                        f"o_{m}", zeros=zeros)

        # ---- V/C logits stream (kvt/kvag/wo freed -> room for vct) ----
        with tc.tile_pool(name="vstream", bufs=32) as vsp, \
             tc.tile_pool(name="vstage", bufs=8) as vst, \
             tc.tile_pool(name="vps", bufs=2, space="PSUM") as vps:
            for g in range(NG):
                if g == 0:
                    vtiles = v0tiles
                else:
                    vtiles = []
                    for k in range(8):
                        vt = vsp.tile([128, GW], BF16, name="vt", tag="vct")
                        nc.sync.dma_start(out=vt, in_=vctp[g, k, :, :])
                        vtiles.append(vt)
                for hh in range(2):
                    stg = vst.tile([128, GW], BF16, name="vstage",
                                   tag="vstage")
                    ps4 = vps.tile([128, 2048], F32, name="ps_v", tag="ps_v")
                    for u in range(4):
                        for k in range(8):
                            nc.tensor.matmul(
                                ps4[:, u * 512: u * 512 + VBW],
                                lhsT=aoT[k][:, hh * 128:(hh + 1) * 128],
                                rhs=vtiles[k][:, u * VBW:(u + 1) * VBW],
                                start=(k == 0), stop=(k == 7))
                    src = ps4.rearrange("p (u w) -> p u w", w=512)[:, :, 0:VBW]
                    dst = stg.rearrange("p (u w) -> p u w", w=VBW)
                    last = (g == NG - 1 and hh == 1)
                    if not last:
                        nc.vector.tensor_copy(out=dst, in_=src)
                        nc.gpsimd.dma_start(out=out[hh, g, :, :], in_=stg)
                    else:
                        # split the final stage so its evacuation + write-out
                        # pipeline instead of serializing after the last MM
                        for u in range(4):
                            nc.vector.tensor_copy(
                                out=dst[:, u, :], in_=src[:, u, :])
                            nc.sync.dma_start(
                                out=out[hh, g, :, u * VBW:(u + 1) * VBW],
                                in_=stg[:, u * VBW:(u + 1) * VBW])


# ---------------- host side ----------------

def _wn_conv(v, g):
    n = np.sqrt((v * v).sum(axis=(1, 2), keepdims=True))
    return g[:, None, None] * v / n


def _wn_lin(v, g):
    return g[:, None] * v / np.linalg.norm(v, axis=1, keepdims=True)


def _selu_np(x):
    return np.where(x > 0, LAM * x,
                    LAM * ALPHA * (np.exp(np.minimum(x, 0)) - 1)).astype(np.float32)


def _bf16(x):
    return np.ascontiguousarray(x.astype(ml_dtypes.bfloat16))


def _f32(x):
    return np.ascontiguousarray(x.astype(np.float32))


_PROGRAM_CACHE = {}


def kernel(o, f, q0_v, q0_g, q0_b, q1_v, q1_g, q1_b,
           out_v, out_g, out_b, V_v, V_g, V_b, C_v, C_g, C_b):
    o, f = np.asarray(o), np.asarray(f)

    w0 = _wn_conv(np.asarray(q0_v), np.asarray(q0_g)) * LAM      # (H, CIN, 3)
    w1 = _wn_conv(np.asarray(q1_v), np.asarray(q1_g)) * LAM      # (H, H, 3)
    b0 = np.asarray(q0_b) * LAM
    b1 = np.asarray(q1_b) * LAM
    woutT = np.ascontiguousarray(_wn_lin(np.asarray(out_v), np.asarray(out_g)).T) * LAM
    outb_l = np.asarray(out_b) * LAM
    vc = np.concatenate([_wn_lin(np.asarray(V_v), np.asarray(V_g)),
                         _wn_lin(np.asarray(C_v), np.asarray(C_g))], axis=0)
    vct = np.ascontiguousarray(vc.T)                             # (H, 32512)
    kv = _selu_np(f)                                             # (S, H)

    # shared packed layouts
    w0T = w0.transpose(2, 1, 0).reshape(30, 128, H)              # (kc, 128, 1024)
    w0pk = _bf16(w0T.reshape(30, 128, 8, 128).transpose(1, 0, 2, 3)
                 .reshape(128, 30 * 1024))
    w1T = w1.transpose(2, 1, 0).reshape(24, 128, H)
    w1pk = _bf16(w1T.reshape(24, 128, 8, 128).transpose(1, 0, 2, 3)
                 .reshape(128, 24 * 1024))
    wopk = _bf16(woutT.reshape(16, 128, 1024).transpose(1, 0, 2)
                 .reshape(128, 16 * 1024))
    vctp = _bf16(vct.reshape(8, 128, NG, GW).transpose(2, 0, 1, 3))
    kvT_full = np.ascontiguousarray(kv.T)                        # (H, S)
    kvtpk = _bf16(kvT_full.reshape(8, 128, S).transpose(1, 0, 2)
                  .reshape(128, 8 * 2048))
    # kvag: [s, (wave-pair, st, hh, 128)]; cols 64..127 = ones, so the
    # mix matmul broadcasts the softmax denominator into psum rows 64..127
    kvag = np.zeros((S, NH, 128), np.float32)
    for n in range(NH):
        kvag[:, n, 0:64] = kv[:, n * 64:(n + 1) * 64]
        kvag[:, n, 64:128] = 1.0
    kvagpk = _bf16(kvag.reshape(16, 128, 8, 2, 128).transpose(1, 2, 0, 3, 4)
                   .reshape(128, 8 * 4096))
    csti = np.zeros((128, 26), np.float32)
    csti[:, 0:8] = b0.reshape(8, 128).T
    csti[:, 8:16] = b1.reshape(8, 128).T
    csti[:, 16:24] = outb_l.reshape(8, 128).T

    # oT with 'same'-conv halo: core i needs o cols [256i-2, 256i+258)
    oT_pad = np.zeros((CIN, L + 4), np.float32)
    oT_pad[:, 2:L + 2] = o.T

    if "nc" not in _PROGRAM_CACHE:
        _PROGRAM_CACHE["nc"] = build_program()
    nc = _PROGRAM_CACHE["nc"]

    in_maps = []
    for i in range(NCORES):
        oTi = oT_pad[:, 256 * i:256 * i + 260]                   # (1280, 260)
        oTpi = _bf16(oTi.reshape(10, 128, 260).transpose(1, 0, 2)
                     .reshape(128, 10 * 260))
        ci = csti.copy()
        ci[:, 24] = 0.0 if i == 0 else 1.0
        ci[:, 25] = 0.0 if i == NCORES - 1 else 1.0
        in_maps.append({
            "oTp": oTpi,
            "w0p": w0pk,
            "w1p": w1pk,
            "kvtp": kvtpk,
            "kvagp": kvagpk,
            "wop": wopk,
            "cst": _f32(ci),
            "vctp": vctp,
        })

    kwargs = {}
    if os.environ.get("NN_COPY_TRACE", "0") == "1":
        kwargs = dict(trace=True)
    res = run_bass_kernel_spmd(nc, in_maps, core_ids=list(range(NCORES)), **kwargs)
    global LAST_RESULTS
    LAST_RESULTS = res
    # core i, local row lr = n*16 + pgl  ->  global row n*128 + 16i + pgl
    full = np.empty((NH, NCORES, 16, VC), np.float32)
    for i in range(NCORES):
        od = np.asarray(res.results[i]["out"]).astype(np.float32)  # (2,NG,128,GW)
        rows = od.transpose(0, 2, 1, 3).reshape(256, VC)           # (lr, VC)
        full[:, i, :, :] = rows.reshape(NH, 16, VC)
    full = full.reshape(L, VC)
    full += np.concatenate([np.asarray(V_b), np.asarray(C_b)])[None, :]
    return full
